# revision 38
# baseline (speedup 1.0000x reference)
"""DeepGraphSAGE (4x SAGEConv + BN/ReLU) on 8 Trainium2 NeuronCores.

Sharding: nodes partitioned across 8 cores (6250 dst nodes each). Each layer:
  - mean-aggregate neighbor features via dma_gather (rows of the allgathered
    H table) + one-hot selection matmuls accumulating in PSUM. The one-hot
    S matrices are built on-chip (iota + is_equal) from an int16 dst table.
  - dense transforms computed in transposed layout (features on partitions)
  - BatchNorm stats via bn_stats/bn_aggr + tiny cross-core AllReduce
  - PE transposes back to row layout, AllGather of H for the next layer.
Hidden-state tables (h1, h2) travel as fp8e3 (E3M4) on the wire and in the
gather table; weights/root terms stay fp16; accumulation/stats are fp32.
"""
import sys
import numpy as np

for p in ("/opt/trn_rl_repo",):
    if p not in sys.path:
        sys.path.append(p)

import concourse.bass as bass
import concourse.bacc as bacc
import concourse.mybir as mybir
from concourse.tile import TileContext
from concourse.masks import make_identity

f32 = mybir.dt.float32
f16 = mybir.dt.float16
fp8 = mybir.dt.float8e3
i16 = mybir.dt.int16

NCORES = 8
P = 128
SPLIT = 32768          # int16 index limit
BASE2 = 17232          # second gather base (recomputed per problem size)
EPS = 1e-5
LAST_BUILD = None
# wire/table dtype of the allgathered hidden state per layer boundary
TAB_DT = {1: fp8, 2: fp8}


# ---------------------------------------------------------------- host prep
class Plan:
    """Per-core gather/selection plan derived from edge_index."""

    def __init__(self, n_nodes, src, dst, core):
        self.n_own = n_nodes // NCORES
        self.nblk = (self.n_own + P - 1) // P
        lo = core * self.n_own
        m = (dst >= lo) & (dst < lo + self.n_own)
        es = src[m].astype(np.int64)
        ed = (dst[m] - lo).astype(np.int64)
        order = np.argsort(ed, kind="stable")
        es, ed = es[order], ed[order]
        bounds = np.searchsorted(ed, np.arange(0, self.nblk + 1) * P)

        idx_vals = []     # flat int16 index stream (multiple of 128 per group)
        dst_vals = []     # per chunk: [128] i16 dst-in-block (-1 pad)
        calls = []        # per PAIR: [(base_id, [k per block in pair]), ...]
        npair = (self.nblk + 1) // 2
        for pr in range(npair):
            blocks = [b for b in (2 * pr, 2 * pr + 1) if b < self.nblk]
            groups = []
            for base_id in (0, 1):
                ks = []
                for b in blocks:
                    e0, e1 = bounds[b], bounds[b + 1]
                    bs, bd = es[e0:e1], ed[e0:e1] - b * P
                    msel = (bs < SPLIT) if base_id == 0 else (bs >= SPLIT)
                    gs, gd = bs[msel], bd[msel]
                    k = (len(gs) + P - 1) // P
                    ks.append(k)
                    if k == 0:
                        continue
                    padded = np.zeros(k * P, np.int64)
                    padded[: len(gs)] = gs - (BASE2 if base_id else 0)
                    idx_vals.append(padded.astype(np.int16))
                    dpad = np.full(k * P, -1, np.int64)
                    dpad[: len(gd)] = gd
                    for j in range(k):
                        dst_vals.append(dpad[j * P:(j + 1) * P].astype(np.int16))
                groups.append((base_id, ks))
            calls.append(groups)

        self.calls = calls
        self.npair = npair
        self.totch = len(dst_vals)
        iv = np.concatenate(idx_vals) if idx_vals else np.zeros(0, np.int16)
        # dma_gather index layout: position i -> [i%16, i//16], replicated 8x
        w = iv.reshape(-1, 16).T if iv.size else np.zeros((16, 0), np.int16)
        self.idx16 = np.tile(w, (8, 1)).copy()           # [128, totch*8] i16
        self.dst16 = np.stack(dst_vals, axis=1).copy() if dst_vals else \
            np.zeros((P, 0), np.int16)                   # [128, totch] i16


def _plan_all(n_nodes, edge_index):
    global BASE2
    BASE2 = max(0, n_nodes - SPLIT)
    src = np.asarray(edge_index[0])
    dst = np.asarray(edge_index[1])
    return [Plan(n_nodes, src, dst, c) for c in range(NCORES)]


# ---------------------------------------------------------------- program
def build_program(n_nodes, in_f, hid, out_f, plan0):
    """One SPMD program (same for all cores; per-core data differs)."""
    nown = plan0.n_own
    nblk = plan0.nblk
    pad_n = nblk * P
    ntile = (nown + 511) // 512
    nfc = hid // P               # 4 feature chunks of the hidden dim
    totch = plan0.totch
    calls = plan0.calls

    nc = bacc.Bacc("TRN2", target_bir_lowering=False, debug=False,
                   num_devices=NCORES, num_swdge_queues=4)

    # ---- I/O ----
    x16 = nc.dram_tensor("x16", [n_nodes, 128], f16, kind="ExternalInput")
    xT = nc.dram_tensor("xT", [in_f, pad_n], f16, kind="ExternalInput")
    idx16_d = nc.dram_tensor("idx16", [P, max(totch * 8, 8)], i16, kind="ExternalInput")
    dst16_d = nc.dram_tensor("dst16", [P, max(totch, 1)], i16, kind="ExternalInput")
    deginv_d = nc.dram_tensor("deginv", [pad_n], f32, kind="ExternalInput")
    wl_d, wr_d, g_d, b_d = {}, {}, {}, {}
    dims = [(in_f, hid), (hid, hid), (hid, hid), (hid, out_f)]
    for l, (fi, fo) in enumerate(dims, start=1):
        wl_d[l] = nc.dram_tensor(f"Wl{l}", [fi, fo], f16, kind="ExternalInput")
        wr_d[l] = nc.dram_tensor(f"Wr{l}", [fi, fo], f16, kind="ExternalInput")
    for l in (1, 2, 3):
        g_d[l] = nc.dram_tensor(f"g{l}", [hid], f32, kind="ExternalInput")
        b_d[l] = nc.dram_tensor(f"b{l}", [hid], f32, kind="ExternalInput")
    bl4_d = nc.dram_tensor("bl4", [out_f], f32, kind="ExternalInput")
    out_d = nc.dram_tensor("out", [nown, out_f], f32, kind="ExternalOutput")

    # ---- internal DRAM ----
    h_own = {l: nc.dram_tensor(f"h{l}_own", [nown, hid], TAB_DT[l]) for l in (1, 2)}
    h_all = {l: nc.dram_tensor(f"h{l}_all", [n_nodes, hid], TAB_DT[l], addr_space="Shared")
             for l in (1, 2)}
    y_own = nc.dram_tensor("y_own", [nown, 128], f16)
    y_all = nc.dram_tensor("y_all", [n_nodes, 128], f16, addr_space="Shared")
    st_in = {l: nc.dram_tensor(f"st{l}_in", [P, 8], f32) for l in (1, 2, 3)}
    st_out = {l: nc.dram_tensor(f"st{l}_out", [P, 8], f32, addr_space="Shared")
              for l in (1, 2, 3)}
    rg = [list(range(NCORES))]

    with TileContext(nc) as tc:
        with (
            tc.tile_pool(name="const", bufs=1) as cp,
            tc.tile_pool(name="sbuf", bufs=2) as sb,
            tc.tile_pool(name="small", bufs=3) as sm,
            tc.tile_pool(name="spool", bufs=2) as sp,
            tc.tile_pool(name="gpool", bufs=3) as gp,
            tc.tile_pool(name="rows", bufs=3) as rp,
            tc.tile_pool(name="psA", bufs=2, space="PSUM") as psA,
            tc.tile_pool(name="psB", bufs=2, space="PSUM") as psB,
            tc.tile_pool(name="psC", bufs=2, space="PSUM") as psC,
        ):
            ident = cp.tile([P, P], f16)
            make_identity(nc, ident[:])
            ident32 = cp.tile([P, P], f32)
            make_identity(nc, ident32[:])
            iota_t = cp.tile([P, P], i16)
            nc.gpsimd.iota(iota_t[:], pattern=[[1, P]], base=0,
                           channel_multiplier=0,
                           allow_small_or_imprecise_dtypes=True)
            deginv_t = cp.tile([P, nblk], f32)
            nc.sync.dma_start(out=deginv_t[:],
                              in_=deginv_d[:].rearrange("(b p) -> p b", p=P))
            # gather indices + dst-in-block tables resident in SBUF
            idxc = cp.tile([P, max(totch * 8, 8)], i16)
            nc.sync.dma_start(out=idxc[:], in_=idx16_d[:, :])
            dstc = cp.tile([P, max(totch, 1)], i16)
            nc.sync.dma_start(out=dstc[:], in_=dst16_d[:, :])
            # weights resident in SBUF, per fi-chunk tiles
            W = {}
            for l, (fi, fo) in enumerate(dims, start=1):
                kc = (fi + P - 1) // P
                for (nm, dram) in (("l", wl_d[l]), ("r", wr_d[l])):
                    for q in range(kc):
                        r0, r1 = q * P, min((q + 1) * P, fi)
                        t = cp.tile([r1 - r0, fo], f16, tag=f"W{nm}{l}_{q}")
                        nc.sync.dma_start(out=t[:], in_=dram[r0:r1, :])
                        W[(nm, l, q)] = t
            gb = {}
            for l in (1, 2, 3):
                for nm, dram in (("g", g_d[l]), ("b", b_d[l])):
                    t = cp.tile([P, nfc], f32, tag=f"{nm}{l}")
                    nc.sync.dma_start(out=t[:], in_=dram[:].rearrange("(c p) -> p c", p=P))
                    gb[(nm, l)] = t
            bl4_t = cp.tile([P, 1], f32)
            nc.sync.dma_start(out=bl4_t[:out_f, :], in_=bl4_d[:, None])

            scr_t = cp.tile([P, 512], f16, tag="scrbuf")
            # persistent hidden state (transposed) + pre-BN buffer
            hT = [cp.tile([P, pad_n], f16, tag=f"hT{q}", name=f"hT{q}") for q in range(nfc)]
            preBN = [cp.tile([P, pad_n], f16, tag=f"preBN{q}", name=f"preBN{q}") for q in range(nfc)]
            if pad_n > nown:
                for q in range(nfc):
                    nc.vector.memset(hT[q][:, nown:pad_n], 0.0)

            gq = [0]  # gather queue round-robin state

            def aggregate_pair(pr, src_table, src_table2, width, tagsfx,
                               row_elems, dt):
                """Mean-aggregate both blocks of pair pr. One dma_gather per
                base-group spanning the pair. Returns list of f16 tiles."""
                groups = calls[pr]
                blocks = [b for b in (2 * pr, 2 * pr + 1) if b < nblk]
                ktot = sum(sum(ks) for _, ks in groups)
                out_tiles = []
                if ktot == 0:
                    for bi in range(len(blocks)):
                        z = sm.tile([P, width], f16, tag=f"agg{tagsfx}{bi}",
                                    name=f"aggz{bi}")
                        nc.vector.memset(z[:], 0.0)
                        out_tiles.append(z)
                    return out_tiles
                ch0 = plan_choff[pr]
                stile = sp.tile([P, ktot, P], dt, tag="S")
                nc.vector.tensor_tensor(
                    out=stile[:],
                    in0=dstc[:, ch0:ch0 + ktot].unsqueeze(2).broadcast_to([P, ktot, P]),
                    in1=iota_t[:].unsqueeze(1).broadcast_to([P, ktot, P]),
                    op=mybir.AluOpType.is_equal,
                )
                g = gp.tile([P, ktot, row_elems], dt, tag="G")
                koff = 0
                for base_id, ks in groups:
                    k = sum(ks)
                    if k == 0:
                        continue
                    src_ap = src_table if base_id == 0 else src_table2
                    nc.gpsimd.dma_gather(
                        out_ap=g[:, koff:koff + k, :],
                        in_ap=src_ap,
                        idxs_ap=idxc[:, (ch0 + koff) * 8:(ch0 + koff + k) * 8],
                        num_idxs=k * P, num_idxs_reg=k * P,
                        elem_size=row_elems, single_packet=False,
                        queue_num=gq[0] % 4,
                    )
                    gq[0] += 1
                    koff += k
                # per-block PSUM accumulation over that block's chunks
                for bi, b in enumerate(blocks):
                    agg_ps = psA.tile([P, 512], f32, tag=f"agg_ps{bi}",
                                      name=f"agg_ps{bi}")
                    mm_idx = []
                    koff = 0
                    for base_id, ks in groups:
                        pre = 0
                        for i2, k2 in enumerate(ks):
                            if i2 == bi:
                                mm_idx += list(range(koff + pre, koff + pre + k2))
                            pre += k2
                        koff += sum(ks)
                    if not mm_idx:
                        z = sm.tile([P, width], f16, tag=f"agg{tagsfx}{bi}",
                                    name=f"aggz2{bi}")
                        nc.vector.memset(z[:], 0.0)
                        out_tiles.append(z)
                        continue
                    for n_, j in enumerate(mm_idx):
                        nc.tensor.matmul(
                            out=agg_ps[:, :width],
                            lhsT=stile[:, j, :], rhs=g[:, j, :width],
                            start=(n_ == 0), stop=(n_ == len(mm_idx) - 1),
                        )
                    asb = sm.tile([P, width], f16, tag=f"agg{tagsfx}{bi}",
                                  name=f"asb{bi}")
                    if b % 2 == 0:
                        nc.vector.tensor_scalar(
                            out=asb[:], in0=agg_ps[:, :width],
                            scalar1=deginv_t[:, b:b + 1], scalar2=None,
                            op0=mybir.AluOpType.mult,
                        )
                    else:
                        nc.scalar.activation(
                            out=asb[:], in_=agg_ps[:, :width],
                            func=mybir.ActivationFunctionType.Copy,
                            scale=deginv_t[:, b:b + 1],
                        )
                    out_tiles.append(asb)
                return out_tiles

            def layer_123(l, src_rows, src_rows2, fi_chunks, rhs_for_fi, width,
                          row_elems, dt, root_pre=False):
                """One SAGE layer with BN+ReLU. rhs_for_fi(q, ns, ne) gives the
                [K, n] rhs AP of the root term for fi-chunk q; aggregation uses
                src_rows tables at `width` features."""
                sums = [sb.tile([P, ntile], f32, tag=f"sums{q}", name=f"sums{q}")
                        for q in range(nfc)]
                sumsqs = [sb.tile([P, ntile], f32, tag=f"sumsq{q}", name=f"sumsq{q}")
                          for q in range(nfc)]
                for nt in range(ntile):
                    ns, ne = nt * 512, min((nt + 1) * 512, nown)
                    nn = ne - ns
                    # aggregate the (up to) 4 dst blocks of this node tile
                    aggT = (sb.tile([in_f, 512], f16, tag="aggT", name="aggT")
                            if width == in_f else None)
                    aggTq = ([sb.tile([P, 512], f16, tag=f"aggT{q}", name=f"aggT{q}")
                              for q in range(fi_chunks)] if width > in_f else None)
                    pair_tiles = []
                    for pr in (2 * nt, 2 * nt + 1):
                        if pr * 2 < nblk:
                            pair_tiles += aggregate_pair(pr, src_rows, src_rows2,
                                                         width, "sb", row_elems, dt)
                    for bi, b in enumerate(range(nt * 4, min(nt * 4 + 4, nblk))):
                        asb = pair_tiles[bi]
                        tp = psB.tile([P, 512], f16, tag="tp")
                        if width == in_f:
                            nc.tensor.matmul(out=tp[:width, bi * P:(bi + 1) * P],
                                             lhsT=asb[:], rhs=ident[:],
                                             is_transpose=True)
                            nc.vector.tensor_copy(out=aggT[:width, bi * P:(bi + 1) * P],
                                                  in_=tp[:width, bi * P:(bi + 1) * P])
                        else:
                            for q in range(fi_chunks):
                                nc.tensor.matmul(out=tp[:, q * P:(q + 1) * P],
                                                 lhsT=asb[:, q * P:(q + 1) * P],
                                                 rhs=ident[:], is_transpose=True)
                                if q % 2 == 0:
                                    nc.vector.tensor_copy(
                                        out=aggTq[q][:, bi * P:(bi + 1) * P],
                                        in_=tp[:, q * P:(q + 1) * P])
                                else:
                                    nc.scalar.activation(
                                        out=aggTq[q][:, bi * P:(bi + 1) * P],
                                        in_=tp[:, q * P:(q + 1) * P],
                                        func=mybir.ActivationFunctionType.Copy)
                    # dense: out^T [fo chunk, nodes]
                    for fo in range(nfc):
                        dps = psC.tile([P, 512], f32, tag="dense")
                        nmm = fi_chunks if root_pre else 2 * fi_chunks
                        mm = 0
                        for q in range(fi_chunks):
                            rhs_agg = (aggT[:width, :nn] if width == in_f
                                       else aggTq[q][:, :nn])
                            nc.tensor.matmul(out=dps[:, :nn],
                                             lhsT=W[("l", l, q)][:, fo * P:(fo + 1) * P],
                                             rhs=rhs_agg, start=(mm == 0),
                                             stop=(mm == nmm - 1))
                            mm += 1
                            if not root_pre:
                                nc.tensor.matmul(out=dps[:, :nn],
                                                 lhsT=W[("r", l, q)][:, fo * P:(fo + 1) * P],
                                                 rhs=rhs_for_fi(q, ns, ne),
                                                 start=False, stop=(mm == nmm - 1))
                                mm += 1
                        if root_pre:
                            # preBN holds the precomputed root term; add agg.
                            nc.vector.scalar_tensor_tensor(
                                out=preBN[fo][:, ns:ne], in0=dps[:, :nn],
                                scalar=1.0, in1=preBN[fo][:, ns:ne],
                                op0=mybir.AluOpType.mult,
                                op1=mybir.AluOpType.add,
                                accum_out=sums[fo][:, nt:nt + 1])
                            nc.scalar.activation(
                                out=scr_t[:, :nn], in_=preBN[fo][:, ns:ne],
                                func=mybir.ActivationFunctionType.Square,
                                accum_out=sumsqs[fo][:, nt:nt + 1])
                        else:
                            nc.scalar.activation(
                                out=scr_t[:, :nn], in_=dps[:, :nn],
                                func=mybir.ActivationFunctionType.Square,
                                accum_out=sumsqs[fo][:, nt:nt + 1])
                            nc.vector.tensor_scalar(
                                out=preBN[fo][:, ns:ne], in0=dps[:, :nn],
                                scalar1=1.0, scalar2=None,
                                op0=mybir.AluOpType.mult,
                                op1=mybir.AluOpType.add,
                                accum_out=sums[fo][:, nt:nt + 1])
                # ---- BN statistics + cross-core allreduce ----
                # pack = per-core [S1, S2] per feature; AllReduce sums them.
                pack = sb.tile([P, 8], f32, tag="pack")
                for q in range(nfc):
                    nc.vector.reduce_sum(out=pack[:, 2 * q:2 * q + 1],
                                         in_=sums[q][:], axis=mybir.AxisListType.X)
                    nc.vector.reduce_sum(out=pack[:, 2 * q + 1:2 * q + 2],
                                         in_=sumsqs[q][:], axis=mybir.AxisListType.X)
                nc.sync.dma_start(out=st_in[l][:, :], in_=pack[:])
                nc.gpsimd.collective_compute(
                    "AllReduce", mybir.AluOpType.add, replica_groups=rg,
                    ins=[st_in[l][:, :]], outs=[st_out[l][:, :]],
                )
                red = sb.tile([P, 8], f32, tag="red")
                nc.sync.dma_start(out=red[:], in_=st_out[l][:, :])
                scale = sb.tile([P, nfc], f32, tag="scale")
                shift = sb.tile([P, nfc], f32, tag="shift")
                inv_n = 1.0 / float(n_nodes)
                for q in range(nfc):
                    mu = sb.tile([P, 1], f32, tag="mu")
                    var = sb.tile([P, 1], f32, tag="var")
                    nc.vector.tensor_scalar(out=mu[:], in0=red[:, 2 * q:2 * q + 1],
                                            scalar1=inv_n, scalar2=None,
                                            op0=mybir.AluOpType.mult)
                    nc.vector.tensor_scalar(out=var[:], in0=red[:, 2 * q + 1:2 * q + 2],
                                            scalar1=inv_n, scalar2=None,
                                            op0=mybir.AluOpType.mult)
                    musq = sb.tile([P, 1], f32, tag="musq")
                    nc.vector.tensor_tensor(out=musq[:], in0=mu[:], in1=mu[:],
                                            op=mybir.AluOpType.mult)
                    nc.vector.tensor_tensor(out=var[:], in0=var[:], in1=musq[:],
                                            op=mybir.AluOpType.subtract)
                    nc.vector.tensor_scalar(out=var[:], in0=var[:], scalar1=EPS,
                                            scalar2=None, op0=mybir.AluOpType.add)
                    nc.vector.reciprocal(out=var[:], in_=var[:])
                    rs = sb.tile([P, 1], f32, tag="rs")
                    nc.scalar.activation(out=rs[:], in_=var[:],
                                         func=mybir.ActivationFunctionType.Sqrt)
                    nc.vector.tensor_tensor(out=scale[:, q:q + 1], in0=rs[:],
                                            in1=gb[("g", l)][:, q:q + 1],
                                            op=mybir.AluOpType.mult)
                    nc.vector.tensor_tensor(out=musq[:], in0=mu[:],
                                            in1=scale[:, q:q + 1],
                                            op=mybir.AluOpType.mult)
                    nc.vector.tensor_tensor(out=shift[:, q:q + 1],
                                            in0=gb[("b", l)][:, q:q + 1], in1=musq[:],
                                            op=mybir.AluOpType.subtract)
                # ---- BN apply + ReLU -> hT (f16), then rows + AllGather ----
                for q in range(nfc):
                    nc.scalar.activation(
                        out=hT[q][:, 0:nown], in_=preBN[q][:, 0:nown],
                        func=mybir.ActivationFunctionType.Relu,
                        bias=shift[:, q:q + 1], scale=scale[:, q:q + 1],
                    )
                if l == 3:
                    return  # h3 is only consumed locally (layer 4 root term)
                for b2 in range(0, nblk, 2):
                    bl2 = [b for b in (b2, b2 + 1) if b < nblk]
                    w2 = len(bl2) * hid
                    tpr = psB.tile([P, 1024], f16, tag="tp")
                    for bi, b in enumerate(bl2):
                        for q in range(nfc):
                            nc.tensor.matmul(
                                out=tpr[:, bi * hid + q * P:bi * hid + (q + 1) * P],
                                lhsT=hT[q][:, b * P:(b + 1) * P],
                                rhs=ident[:], is_transpose=True)
                    rows = rp.tile([P, 1024], TAB_DT[l], tag="rows")
                    if (b2 // 2) % 2 == 0:
                        nc.vector.tensor_copy(out=rows[:, :w2], in_=tpr[:, :w2])
                    else:
                        nc.scalar.activation(
                            out=rows[:, :w2], in_=tpr[:, :w2],
                            func=mybir.ActivationFunctionType.Copy)
                    for bi, b in enumerate(bl2):
                        ns, ne = b * P, min((b + 1) * P, nown)
                        nc.sync.dma_start(
                            out=h_own[l][ns:ne, :],
                            in_=rows[:ne - ns, bi * hid:bi * hid + hid])
                nc.gpsimd.collective_compute(
                    "AllGather", mybir.AluOpType.bypass, replica_groups=rg,
                    ins=[h_own[l][:, :]], outs=[h_all[l][:, :]],
                )
                # precompute next layer's root term (Wr_{l+1}^T @ hT) into
                # preBN; runs on PE while the AllGather moves bytes
                nl = l + 1
                for nt2 in range(ntile):
                    ns2, ne2 = nt2 * 512, min((nt2 + 1) * 512, nown)
                    nn2 = ne2 - ns2
                    for fo in range(nfc):
                        rps = psC.tile([P, 512], f32, tag="dense")
                        for q in range(nfc):
                            nc.tensor.matmul(
                                out=rps[:, :nn2],
                                lhsT=W[("r", nl, q)][:, fo * P:(fo + 1) * P],
                                rhs=hT[q][:, ns2:ne2],
                                start=(q == 0), stop=(q == nfc - 1))
                        if (nt2 + fo) % 2 == 0:
                            nc.vector.tensor_copy(out=preBN[fo][:, ns2:ne2],
                                                  in_=rps[:, :nn2])
                        else:
                            nc.scalar.activation(
                                out=preBN[fo][:, ns2:ne2], in_=rps[:, :nn2],
                                func=mybir.ActivationFunctionType.Copy)

            # ================= layer 1 =================
            def xT_rhs(q, ns, ne):
                xt = sm.tile([in_f, 512], f16, tag="xTt", name="xTt")
                nc.sync.dma_start(out=xt[:, :ne - ns], in_=xT[:, ns:ne])
                return xt[:, :ne - ns]
            layer_123(1, x16[:, :], x16[BASE2:, :], 1, xT_rhs, in_f, 128, f16)
            # ================= layers 2,3 =================
            for l in (2, 3):
                layer_123(l, h_all[l - 1][:, :], h_all[l - 1][BASE2:, :], nfc,
                          None, hid, hid, TAB_DT[l - 1], root_pre=True)
            # ================= layer 4 =================
            # y = h3 @ Wl4 (transposed), to rows, allgather
            for nt in range(ntile):
                ns, ne = nt * 512, min((nt + 1) * 512, nown)
                nn = ne - ns
                yps = psC.tile([P, 512], f32, tag="dense")
                for q in range(nfc):
                    nc.tensor.matmul(out=yps[:out_f, :nn],
                                     lhsT=W[("l", 4, q)][:, :out_f],
                                     rhs=hT[q][:, ns:ne],
                                     start=(q == 0), stop=(q == nfc - 1))
                ysb = sb.tile([P, 512], f16, tag="ysb")
                nc.vector.tensor_copy(out=ysb[:out_f, :nn], in_=yps[:out_f, :nn])
                for bi in range((nn + P - 1) // P):
                    b0 = bi * P
                    b1 = min(b0 + P, nn)
                    tpy = psB.tile([P, 512], f16, tag="tp")
                    nc.tensor.matmul(out=tpy[:b1 - b0, :out_f],
                                     lhsT=ysb[:out_f, b0:b1],
                                     rhs=ident[:out_f, :out_f],
                                     is_transpose=True)
                    yr = sb.tile([P, 128], f16, tag="yrows")
                    nc.vector.memset(yr[:], 0.0)
                    if bi % 2 == 0:
                        nc.vector.tensor_copy(out=yr[:b1 - b0, :out_f],
                                              in_=tpy[:b1 - b0, :out_f])
                    else:
                        nc.scalar.activation(out=yr[:b1 - b0, :out_f],
                                             in_=tpy[:b1 - b0, :out_f],
                                             func=mybir.ActivationFunctionType.Copy)
                    nc.sync.dma_start(out=y_own[ns + b0:ns + b1, :],
                                      in_=yr[:b1 - b0, :])
            for nt in range(ntile):
                ns, ne = nt * 512, min((nt + 1) * 512, nown)
                nn = ne - ns
                rps4 = psC.tile([P, 512], f32, tag="dense")
                for q in range(nfc):
                    nc.tensor.matmul(out=rps4[:out_f, :nn],
                                     lhsT=W[("r", 4, q)][:, :out_f],
                                     rhs=hT[q][:, ns:ne],
                                     start=(q == 0), stop=(q == nfc - 1))
                if nt % 2 == 0:
                    nc.vector.tensor_copy(out=preBN[0][:out_f, ns:ne],
                                          in_=rps4[:out_f, :nn])
                else:
                    nc.scalar.activation(out=preBN[0][:out_f, ns:ne],
                                         in_=rps4[:out_f, :nn],
                                         func=mybir.ActivationFunctionType.Copy)
            nc.gpsimd.collective_compute(
                "AllGather", mybir.AluOpType.bypass, replica_groups=rg,
                ins=[y_own[:, :]], outs=[y_all[:, :]],
            )
            # final: out = mean-agg(y) + h3 @ Wr4 + bl4
            for nt in range(ntile):
                ns, ne = nt * 512, min((nt + 1) * 512, nown)
                nn = ne - ns
                agg4T = sb.tile([P, 512], f16, tag="agg4T")
                pair_tiles4 = []
                for pr in (2 * nt, 2 * nt + 1):
                    if pr * 2 < nblk:
                        pair_tiles4 += aggregate_pair(pr, y_all[:, :],
                                                      y_all[BASE2:, :],
                                                      out_f, "4", 128, f16)
                for bi, b in enumerate(range(nt * 4, min(nt * 4 + 4, nblk))):
                    asb = pair_tiles4[bi]
                    tp = psB.tile([P, 512], f16, tag="tp")
                    nc.tensor.matmul(out=tp[:out_f, bi * P:(bi + 1) * P],
                                     lhsT=asb[:], rhs=ident[:], is_transpose=True)
                    nc.vector.tensor_copy(out=agg4T[:out_f, bi * P:(bi + 1) * P],
                                          in_=tp[:out_f, bi * P:(bi + 1) * P])
                osb = sb.tile([P, 512], f32, tag="osb")
                nc.vector.tensor_tensor(out=osb[:out_f, :nn],
                                        in0=preBN[0][:out_f, ns:ne],
                                        in1=agg4T[:out_f, :nn],
                                        op=mybir.AluOpType.add)
                nc.vector.tensor_scalar(out=osb[:out_f, :nn], in0=osb[:out_f, :nn],
                                        scalar1=bl4_t[:out_f, 0:1], scalar2=None,
                                        op0=mybir.AluOpType.add)
                for bi in range((nn + P - 1) // P):
                    b0, b1 = bi * P, min(bi * P + P, nn)
                    tpo = psB.tile([P, 512], f32, tag="tp")
                    nc.tensor.matmul(out=tpo[:b1 - b0, :out_f],
                                     lhsT=osb[:out_f, b0:b1],
                                     rhs=ident32[:out_f, :out_f],
                                     is_transpose=True)
                    orow = sb.tile([P, out_f], f32, tag="orow")
                    nc.vector.tensor_copy(out=orow[:b1 - b0, :],
                                          in_=tpo[:b1 - b0, :out_f])
                    nc.sync.dma_start(out=out_d[ns + b0:ns + b1, :],
                                      in_=orow[:b1 - b0, :])
    return nc


# chunk offsets per block, filled by build_inputs (shared plan state)
plan_choff = []


def _prep(plan):
    """Fill global chunk-offset table for the builder."""
    global plan_choff
    plan_choff = []
    off = 0
    for groups in plan.calls:
        plan_choff.append(off)
        off += sum(sum(ks) for _, ks in groups)


def _execute(nc, in_maps):
    from concourse.bass_utils import run_bass_kernel_spmd
    res = run_bass_kernel_spmd(nc, in_maps, list(range(NCORES)))
    return [res.results[c] for c in range(NCORES)]


def kernel(**inputs):
    x = np.asarray(inputs["x"], np.float32)
    edge_index = np.asarray(inputs["edge_index"])
    n_nodes, in_f = x.shape
    hid = inputs["Wl2"].shape[0]
    out_f = inputs["Wl4"].shape[1]
    nown = n_nodes // NCORES

    src = np.asarray(edge_index[0]).astype(np.int64)
    dst = np.asarray(edge_index[1]).astype(np.int64)
    deg = np.bincount(dst, minlength=n_nodes).astype(np.float32)
    deginv = (1.0 / np.maximum(deg, 1.0)).astype(np.float32)

    plans = _plan_all(n_nodes, edge_index)
    # pad chunk counts to the max across cores so one program fits all
    plans = _pad_plans(plans)
    _prep(plans[0])

    import time as _time
    _t0 = _time.perf_counter()
    nc = build_program(n_nodes, in_f, hid, out_f, plans[0])
    print(f"[kernel] program built in {_time.perf_counter() - _t0:.1f}s", flush=True)
    _t0 = _time.perf_counter()
    nc.compile()
    print(f"[kernel] bacc compile in {_time.perf_counter() - _t0:.1f}s", flush=True)

    x16 = np.zeros((n_nodes, 128), np.float16)
    x16[:, :in_f] = x.astype(np.float16)
    nblk = plans[0].nblk
    pad_n = nblk * P

    in_maps = []
    for c, p in enumerate(plans):
        xTc = np.zeros((in_f, pad_n), np.float16)
        xTc[:, :nown] = x[c * nown:(c + 1) * nown].T.astype(np.float16)
        dg = np.zeros(pad_n, np.float32)
        dg[:nown] = deginv[c * nown:(c + 1) * nown]
        im = {
            "x16": x16, "xT": xTc,
            "idx16": p.idx16 if p.idx16.size else np.zeros((P, 8), np.int16),
            "dst16": p.dst16 if p.dst16.size else np.zeros((P, 1), np.int16),
            "deginv": dg,
            "bl4": np.asarray(inputs["bl4"], np.float32),
        }
        for l in (1, 2, 3, 4):
            im[f"Wl{l}"] = np.asarray(inputs[f"Wl{l}"], np.float16)
            im[f"Wr{l}"] = np.asarray(inputs[f"Wr{l}"], np.float16)
        for l in (1, 2, 3):
            im[f"g{l}"] = np.asarray(inputs[f"g{l}"], np.float32)
            im[f"b{l}"] = np.asarray(inputs[f"b{l}"], np.float32)
        in_maps.append(im)

    global LAST_BUILD
    LAST_BUILD = (nc, in_maps)
    results = _execute(nc, in_maps)
    out = np.concatenate([results[c]["out"] for c in range(NCORES)], axis=0)
    return out.astype(np.float32)


_f8np = mybir.dt.np(fp8)


def _build_s(p):
    """One-hot S chunks [128, totch, 128] f16 from the dst16 table."""
    if p.dst16.size == 0 or p.totch == 0:
        return np.zeros((P, 1, P), np.float16)
    S = np.zeros((P, p.totch, P), np.float16)
    e = np.arange(P)
    for ch in range(p.totch):
        d = p.dst16[:, ch]
        m = d >= 0
        S[e[m], ch, d[m]] = 1.0
    return S


def _pad_plans(plans):
    """Pad every core's per-(block,group) chunk count to the cross-core max
    and rebuild idx16/dst16 accordingly, so one program serves all cores."""
    npair = plans[0].npair
    kmax = {}
    for pr in range(npair):
        for gi in range(2):
            nb = len(plans[0].calls[pr][gi][1])
            kmax[(pr, gi)] = [max(p.calls[pr][gi][1][i] for p in plans)
                              for i in range(nb)]
    for p in plans:
        idx_vals, dst_vals, calls = [], [], []
        off = 0
        orig_iv = _unwrap_idx(p.idx16, p.totch)
        for pr in range(npair):
            groups = []
            for gi in range(2):
                base_id, ks = p.calls[pr][gi]
                kms = kmax[(pr, gi)]
                for i, (k, km) in enumerate(zip(ks, kms)):
                    iv = np.zeros(km * P, np.int16)
                    dv = np.full((P, km), -1, np.int16)
                    if k:
                        iv[:k * P] = orig_iv[off * P:(off + k) * P]
                        dv[:, :k] = p.dst16[:, off:off + k]
                    off += k
                    idx_vals.append(iv)
                    dst_vals.append(dv)
                groups.append((base_id, list(kms)))
            calls.append(groups)
        p.calls = calls
        p.totch = sum(sum(kmax[(pr, gi)]) for pr in range(npair) for gi in range(2))
        iv = np.concatenate(idx_vals) if idx_vals else np.zeros(0, np.int16)
        w = iv.reshape(-1, 16).T if iv.size else np.zeros((16, 0), np.int16)
        p.idx16 = np.tile(w, (8, 1)).copy()
        p.dst16 = np.concatenate(dst_vals, axis=1).copy() if dst_vals else \
            np.zeros((P, 1), np.int16)
    return plans


def _unwrap_idx(idx16, totch):
    """Inverse of the 16-partition wrap: [128, totch*8] -> flat [totch*128]."""
    if idx16.size == 0:
        return np.zeros(0, np.int16)
    return idx16[:16, :].T.reshape(-1)


# revision 40
# speedup vs baseline: 1.0491x; 1.0491x over previous
"""DeepGraphSAGE (4x SAGEConv + BN/ReLU) on 8 Trainium2 NeuronCores.

Sharding: nodes partitioned across 8 cores (6250 dst nodes each). Each layer:
  - mean-aggregate neighbor features via dma_gather (rows of the allgathered
    H table) + one-hot selection matmuls accumulating in PSUM. The one-hot
    S matrices are built on-chip (iota + is_equal) from an int16 dst table.
  - dense transforms computed in transposed layout (features on partitions)
  - BatchNorm stats via bn_stats/bn_aggr + tiny cross-core AllReduce
  - PE transposes back to row layout, AllGather of H for the next layer.
Hidden-state tables (h1, h2) travel as fp8e3 (E3M4) on the wire and in the
gather table; weights/root terms stay fp16; accumulation/stats are fp32.
"""
import sys
import numpy as np

for p in ("/opt/trn_rl_repo",):
    if p not in sys.path:
        sys.path.append(p)

import concourse.bass as bass
import concourse.bacc as bacc
import concourse.mybir as mybir
from concourse.tile import TileContext
from concourse.masks import make_identity

f32 = mybir.dt.float32
f16 = mybir.dt.float16
fp8 = mybir.dt.float8e3
i16 = mybir.dt.int16

NCORES = 8
P = 128
SPLIT = 32768          # int16 index limit
BASE2 = 17232          # second gather base (recomputed per problem size)
EPS = 1e-5
LAST_BUILD = None
# wire/table dtype of the allgathered hidden state per layer boundary
TAB_DT = {1: fp8, 2: fp8}


# ---------------------------------------------------------------- host prep
class Plan:
    """Per-core gather/selection plan derived from edge_index."""

    def __init__(self, n_nodes, src, dst, core):
        self.n_own = n_nodes // NCORES
        self.nblk = (self.n_own + P - 1) // P
        lo = core * self.n_own
        m = (dst >= lo) & (dst < lo + self.n_own)
        es = src[m].astype(np.int64)
        ed = (dst[m] - lo).astype(np.int64)
        order = np.argsort(ed, kind="stable")
        es, ed = es[order], ed[order]
        bounds = np.searchsorted(ed, np.arange(0, self.nblk + 1) * P)

        idx_vals = []     # flat int16 index stream (multiple of 128 per group)
        dst_vals = []     # per chunk: [128] i16 dst-in-block (-1 pad)
        calls = []        # per PAIR: [(base_id, [k per block in pair]), ...]
        npair = (self.nblk + 1) // 2
        for pr in range(npair):
            blocks = [b for b in (2 * pr, 2 * pr + 1) if b < self.nblk]
            groups = []
            for base_id in (0, 1):
                ks = []
                for b in blocks:
                    e0, e1 = bounds[b], bounds[b + 1]
                    bs, bd = es[e0:e1], ed[e0:e1] - b * P
                    msel = (bs < SPLIT) if base_id == 0 else (bs >= SPLIT)
                    gs, gd = bs[msel], bd[msel]
                    k = (len(gs) + P - 1) // P
                    ks.append(k)
                    if k == 0:
                        continue
                    padded = np.zeros(k * P, np.int64)
                    padded[: len(gs)] = gs - (BASE2 if base_id else 0)
                    idx_vals.append(padded.astype(np.int16))
                    dpad = np.full(k * P, -1, np.int64)
                    dpad[: len(gd)] = gd
                    for j in range(k):
                        dst_vals.append(dpad[j * P:(j + 1) * P].astype(np.int16))
                groups.append((base_id, ks))
            calls.append(groups)

        self.calls = calls
        self.npair = npair
        self.totch = len(dst_vals)
        iv = np.concatenate(idx_vals) if idx_vals else np.zeros(0, np.int16)
        # dma_gather index layout: position i -> [i%16, i//16], replicated 8x
        w = iv.reshape(-1, 16).T if iv.size else np.zeros((16, 0), np.int16)
        self.idx16 = np.tile(w, (8, 1)).copy()           # [128, totch*8] i16
        self.dst16 = np.stack(dst_vals, axis=1).copy() if dst_vals else \
            np.zeros((P, 0), np.int16)                   # [128, totch] i16


def _plan_all(n_nodes, edge_index):
    global BASE2
    BASE2 = max(0, n_nodes - SPLIT)
    src = np.asarray(edge_index[0])
    dst = np.asarray(edge_index[1])
    return [Plan(n_nodes, src, dst, c) for c in range(NCORES)]


# ---------------------------------------------------------------- program
def build_program(n_nodes, in_f, hid, out_f, plan0):
    """One SPMD program (same for all cores; per-core data differs)."""
    nown = plan0.n_own
    nblk = plan0.nblk
    pad_n = nblk * P
    ntile = (nown + 511) // 512
    nfc = hid // P               # 4 feature chunks of the hidden dim
    totch = plan0.totch
    calls = plan0.calls

    nc = bacc.Bacc("TRN2", target_bir_lowering=False, debug=False,
                   num_devices=NCORES, num_swdge_queues=4)

    # ---- I/O ----
    x16 = nc.dram_tensor("x16", [n_nodes, 128], f16, kind="ExternalInput")
    xT = nc.dram_tensor("xT", [in_f, pad_n], f16, kind="ExternalInput")
    idx16_d = nc.dram_tensor("idx16", [P, max(totch * 8, 8)], i16, kind="ExternalInput")
    dst16_d = nc.dram_tensor("dst16", [P, max(totch, 1)], i16, kind="ExternalInput")
    deginv_d = nc.dram_tensor("deginv", [pad_n], f32, kind="ExternalInput")
    wl_d, wr_d, g_d, b_d = {}, {}, {}, {}
    dims = [(in_f, hid), (hid, hid), (hid, hid), (hid, out_f)]
    for l, (fi, fo) in enumerate(dims, start=1):
        wl_d[l] = nc.dram_tensor(f"Wl{l}", [fi, fo], f16, kind="ExternalInput")
        wr_d[l] = nc.dram_tensor(f"Wr{l}", [fi, fo], f16, kind="ExternalInput")
    for l in (1, 2, 3):
        g_d[l] = nc.dram_tensor(f"g{l}", [hid], f32, kind="ExternalInput")
        b_d[l] = nc.dram_tensor(f"b{l}", [hid], f32, kind="ExternalInput")
    bl4_d = nc.dram_tensor("bl4", [out_f], f32, kind="ExternalInput")
    out_d = nc.dram_tensor("out", [nown, out_f], f32, kind="ExternalOutput")

    # ---- internal DRAM ----
    h_own = {l: nc.dram_tensor(f"h{l}_own", [nown, hid], TAB_DT[l]) for l in (1, 2)}
    h_all = {l: nc.dram_tensor(f"h{l}_all", [n_nodes, hid], TAB_DT[l], addr_space="Shared")
             for l in (1, 2)}
    y_own = nc.dram_tensor("y_own", [nown, 128], f16)
    y_all = nc.dram_tensor("y_all", [n_nodes, 128], f16, addr_space="Shared")
    st_in = {l: nc.dram_tensor(f"st{l}_in", [P, 8], f32) for l in (1, 2, 3)}
    st_out = {l: nc.dram_tensor(f"st{l}_out", [P, 8], f32, addr_space="Shared")
              for l in (1, 2, 3)}
    rg = [list(range(NCORES))]

    with TileContext(nc) as tc:
        with (
            tc.tile_pool(name="const", bufs=1) as cp,
            tc.tile_pool(name="sbuf", bufs=2) as sb,
            tc.tile_pool(name="small", bufs=3) as sm,
            tc.tile_pool(name="spool", bufs=2) as sp,
            tc.tile_pool(name="gpool", bufs=3) as gp,
            tc.tile_pool(name="rows", bufs=3) as rp,
            tc.tile_pool(name="psA", bufs=2, space="PSUM") as psA,
            tc.tile_pool(name="psB", bufs=2, space="PSUM") as psB,
            tc.tile_pool(name="psC", bufs=2, space="PSUM") as psC,
        ):
            ident = cp.tile([P, P], f16)
            make_identity(nc, ident[:])
            ident32 = cp.tile([P, P], f32)
            make_identity(nc, ident32[:])
            iota_t = cp.tile([P, P], i16)
            nc.gpsimd.iota(iota_t[:], pattern=[[1, P]], base=0,
                           channel_multiplier=0,
                           allow_small_or_imprecise_dtypes=True)
            deginv_t = cp.tile([P, nblk], f32)
            nc.sync.dma_start(out=deginv_t[:],
                              in_=deginv_d[:].rearrange("(b p) -> p b", p=P))
            # gather indices + dst-in-block tables resident in SBUF
            idxc = cp.tile([P, max(totch * 8, 8)], i16)
            nc.sync.dma_start(out=idxc[:], in_=idx16_d[:, :])
            dstc = cp.tile([P, max(totch, 1)], i16)
            nc.sync.dma_start(out=dstc[:], in_=dst16_d[:, :])
            # weights resident in SBUF, per fi-chunk tiles
            W = {}
            for l, (fi, fo) in enumerate(dims, start=1):
                kc = (fi + P - 1) // P
                for (nm, dram) in (("l", wl_d[l]), ("r", wr_d[l])):
                    for q in range(kc):
                        r0, r1 = q * P, min((q + 1) * P, fi)
                        t = cp.tile([r1 - r0, fo], f16, tag=f"W{nm}{l}_{q}")
                        nc.sync.dma_start(out=t[:], in_=dram[r0:r1, :])
                        W[(nm, l, q)] = t
            gb = {}
            for l in (1, 2, 3):
                for nm, dram in (("g", g_d[l]), ("b", b_d[l])):
                    t = cp.tile([P, nfc], f32, tag=f"{nm}{l}")
                    nc.sync.dma_start(out=t[:], in_=dram[:].rearrange("(c p) -> p c", p=P))
                    gb[(nm, l)] = t
            bl4_t = cp.tile([P, 1], f32)
            nc.sync.dma_start(out=bl4_t[:out_f, :], in_=bl4_d[:, None])

            # persistent hidden state (transposed) + pre-BN buffer
            hT = [cp.tile([P, pad_n], f16, tag=f"hT{q}", name=f"hT{q}") for q in range(nfc)]
            preBN = [cp.tile([P, pad_n], f16, tag=f"preBN{q}", name=f"preBN{q}") for q in range(nfc)]
            if pad_n > nown:
                for q in range(nfc):
                    nc.vector.memset(hT[q][:, nown:pad_n], 0.0)

            gq = [0]  # gather queue round-robin state

            def aggregate_pair(pr, src_table, src_table2, width, tagsfx,
                               row_elems, dt):
                """Mean-aggregate both blocks of pair pr. One dma_gather per
                base-group spanning the pair. Returns list of f16 tiles."""
                groups = calls[pr]
                blocks = [b for b in (2 * pr, 2 * pr + 1) if b < nblk]
                ktot = sum(sum(ks) for _, ks in groups)
                out_tiles = []
                if ktot == 0:
                    for bi in range(len(blocks)):
                        z = sm.tile([P, width], f16, tag=f"agg{tagsfx}{bi}",
                                    name=f"aggz{bi}")
                        nc.vector.memset(z[:], 0.0)
                        out_tiles.append(z)
                    return out_tiles
                ch0 = plan_choff[pr]
                stile = sp.tile([P, ktot, P], dt, tag="S")
                nc.vector.tensor_tensor(
                    out=stile[:],
                    in0=dstc[:, ch0:ch0 + ktot].unsqueeze(2).broadcast_to([P, ktot, P]),
                    in1=iota_t[:].unsqueeze(1).broadcast_to([P, ktot, P]),
                    op=mybir.AluOpType.is_equal,
                )
                g = gp.tile([P, ktot, row_elems], dt, tag="G")
                koff = 0
                for base_id, ks in groups:
                    k = sum(ks)
                    if k == 0:
                        continue
                    src_ap = src_table if base_id == 0 else src_table2
                    nc.gpsimd.dma_gather(
                        out_ap=g[:, koff:koff + k, :],
                        in_ap=src_ap,
                        idxs_ap=idxc[:, (ch0 + koff) * 8:(ch0 + koff + k) * 8],
                        num_idxs=k * P, num_idxs_reg=k * P,
                        elem_size=row_elems, single_packet=False,
                        queue_num=gq[0] % 4,
                    )
                    gq[0] += 1
                    koff += k
                # per-block PSUM accumulation over that block's chunks
                for bi, b in enumerate(blocks):
                    agg_ps = psA.tile([P, 512], f32, tag=f"agg_ps{bi}",
                                      name=f"agg_ps{bi}")
                    mm_idx = []
                    koff = 0
                    for base_id, ks in groups:
                        pre = 0
                        for i2, k2 in enumerate(ks):
                            if i2 == bi:
                                mm_idx += list(range(koff + pre, koff + pre + k2))
                            pre += k2
                        koff += sum(ks)
                    if not mm_idx:
                        z = sm.tile([P, width], f16, tag=f"agg{tagsfx}{bi}",
                                    name=f"aggz2{bi}")
                        nc.vector.memset(z[:], 0.0)
                        out_tiles.append(z)
                        continue
                    for n_, j in enumerate(mm_idx):
                        nc.tensor.matmul(
                            out=agg_ps[:, :width],
                            lhsT=stile[:, j, :], rhs=g[:, j, :width],
                            start=(n_ == 0), stop=(n_ == len(mm_idx) - 1),
                        )
                    asb = sm.tile([P, width], f16, tag=f"agg{tagsfx}{bi}",
                                  name=f"asb{bi}")
                    nc.vector.tensor_scalar(
                        out=asb[:], in0=agg_ps[:, :width],
                        scalar1=deginv_t[:, b:b + 1], scalar2=None,
                        op0=mybir.AluOpType.mult,
                    )
                    out_tiles.append(asb)
                return out_tiles

            def layer_123(l, src_rows, src_rows2, fi_chunks, rhs_for_fi, width,
                          row_elems, dt, root_pre=False):
                """One SAGE layer with BN+ReLU. rhs_for_fi(q, ns, ne) gives the
                [K, n] rhs AP of the root term for fi-chunk q; aggregation uses
                src_rows tables at `width` features."""
                sums = [sb.tile([P, ntile], f32, tag=f"sums{q}", name=f"sums{q}")
                        for q in range(nfc)]
                sumsqs = [sb.tile([P, ntile], f32, tag=f"sumsq{q}", name=f"sumsq{q}")
                          for q in range(nfc)]
                for nt in range(ntile):
                    ns, ne = nt * 512, min((nt + 1) * 512, nown)
                    nn = ne - ns
                    # aggregate the (up to) 4 dst blocks of this node tile
                    aggT = (sb.tile([in_f, 512], f16, tag="aggT", name="aggT")
                            if width == in_f else None)
                    aggTq = ([sb.tile([P, 512], f16, tag=f"aggT{q}", name=f"aggT{q}")
                              for q in range(fi_chunks)] if width > in_f else None)
                    pair_tiles = []
                    for pr in (2 * nt, 2 * nt + 1):
                        if pr * 2 < nblk:
                            pair_tiles += aggregate_pair(pr, src_rows, src_rows2,
                                                         width, "sb", row_elems, dt)
                    for bi, b in enumerate(range(nt * 4, min(nt * 4 + 4, nblk))):
                        asb = pair_tiles[bi]
                        tp = psB.tile([P, 512], f16, tag="tp")
                        if width == in_f:
                            nc.tensor.matmul(out=tp[:width, bi * P:(bi + 1) * P],
                                             lhsT=asb[:], rhs=ident[:],
                                             is_transpose=True)
                            nc.vector.tensor_copy(out=aggT[:width, bi * P:(bi + 1) * P],
                                                  in_=tp[:width, bi * P:(bi + 1) * P])
                        else:
                            for q in range(fi_chunks):
                                nc.tensor.matmul(out=tp[:, q * P:(q + 1) * P],
                                                 lhsT=asb[:, q * P:(q + 1) * P],
                                                 rhs=ident[:], is_transpose=True)
                                nc.vector.tensor_copy(out=aggTq[q][:, bi * P:(bi + 1) * P],
                                                      in_=tp[:, q * P:(q + 1) * P])
                    # dense: out^T [fo chunk, nodes]
                    for fo in range(nfc):
                        dps = psC.tile([P, 512], f32, tag="dense")
                        nmm = fi_chunks if root_pre else 2 * fi_chunks
                        mm = 0
                        for q in range(fi_chunks):
                            rhs_agg = (aggT[:width, :nn] if width == in_f
                                       else aggTq[q][:, :nn])
                            nc.tensor.matmul(out=dps[:, :nn],
                                             lhsT=W[("l", l, q)][:, fo * P:(fo + 1) * P],
                                             rhs=rhs_agg, start=(mm == 0),
                                             stop=(mm == nmm - 1))
                            mm += 1
                            if not root_pre:
                                nc.tensor.matmul(out=dps[:, :nn],
                                                 lhsT=W[("r", l, q)][:, fo * P:(fo + 1) * P],
                                                 rhs=rhs_for_fi(q, ns, ne),
                                                 start=False, stop=(mm == nmm - 1))
                                mm += 1
                        if root_pre:
                            # preBN holds the precomputed root term; add agg.
                            nc.vector.scalar_tensor_tensor(
                                out=preBN[fo][:, ns:ne], in0=dps[:, :nn],
                                scalar=1.0, in1=preBN[fo][:, ns:ne],
                                op0=mybir.AluOpType.mult,
                                op1=mybir.AluOpType.add,
                                accum_out=sums[fo][:, nt:nt + 1])
                            scr = sm.tile([P, 512], f16, tag="scr", name="scr")
                            nc.scalar.activation(
                                out=scr[:, :nn], in_=preBN[fo][:, ns:ne],
                                func=mybir.ActivationFunctionType.Square,
                                accum_out=sumsqs[fo][:, nt:nt + 1])
                        else:
                            scr = sm.tile([P, 512], f16, tag="scr", name="scr")
                            nc.scalar.activation(
                                out=scr[:, :nn], in_=dps[:, :nn],
                                func=mybir.ActivationFunctionType.Square,
                                accum_out=sumsqs[fo][:, nt:nt + 1])
                            nc.vector.tensor_scalar(
                                out=preBN[fo][:, ns:ne], in0=dps[:, :nn],
                                scalar1=1.0, scalar2=None,
                                op0=mybir.AluOpType.mult,
                                op1=mybir.AluOpType.add,
                                accum_out=sums[fo][:, nt:nt + 1])
                # ---- BN statistics + cross-core allreduce ----
                # pack = per-core [S1, S2] per feature; AllReduce sums them.
                pack = sb.tile([P, 8], f32, tag="pack")
                for q in range(nfc):
                    nc.vector.reduce_sum(out=pack[:, 2 * q:2 * q + 1],
                                         in_=sums[q][:], axis=mybir.AxisListType.X)
                    nc.vector.reduce_sum(out=pack[:, 2 * q + 1:2 * q + 2],
                                         in_=sumsqs[q][:], axis=mybir.AxisListType.X)
                nc.sync.dma_start(out=st_in[l][:, :], in_=pack[:])
                nc.gpsimd.collective_compute(
                    "AllReduce", mybir.AluOpType.add, replica_groups=rg,
                    ins=[st_in[l][:, :]], outs=[st_out[l][:, :]],
                )
                red = sb.tile([P, 8], f32, tag="red")
                nc.sync.dma_start(out=red[:], in_=st_out[l][:, :])
                scale = sb.tile([P, nfc], f32, tag="scale")
                shift = sb.tile([P, nfc], f32, tag="shift")
                inv_n = 1.0 / float(n_nodes)
                for q in range(nfc):
                    mu = sb.tile([P, 1], f32, tag="mu")
                    var = sb.tile([P, 1], f32, tag="var")
                    nc.vector.tensor_scalar(out=mu[:], in0=red[:, 2 * q:2 * q + 1],
                                            scalar1=inv_n, scalar2=None,
                                            op0=mybir.AluOpType.mult)
                    nc.vector.tensor_scalar(out=var[:], in0=red[:, 2 * q + 1:2 * q + 2],
                                            scalar1=inv_n, scalar2=None,
                                            op0=mybir.AluOpType.mult)
                    musq = sb.tile([P, 1], f32, tag="musq")
                    nc.vector.tensor_tensor(out=musq[:], in0=mu[:], in1=mu[:],
                                            op=mybir.AluOpType.mult)
                    nc.vector.tensor_tensor(out=var[:], in0=var[:], in1=musq[:],
                                            op=mybir.AluOpType.subtract)
                    nc.vector.tensor_scalar(out=var[:], in0=var[:], scalar1=EPS,
                                            scalar2=None, op0=mybir.AluOpType.add)
                    nc.vector.reciprocal(out=var[:], in_=var[:])
                    rs = sb.tile([P, 1], f32, tag="rs")
                    nc.scalar.activation(out=rs[:], in_=var[:],
                                         func=mybir.ActivationFunctionType.Sqrt)
                    nc.vector.tensor_tensor(out=scale[:, q:q + 1], in0=rs[:],
                                            in1=gb[("g", l)][:, q:q + 1],
                                            op=mybir.AluOpType.mult)
                    nc.vector.tensor_tensor(out=musq[:], in0=mu[:],
                                            in1=scale[:, q:q + 1],
                                            op=mybir.AluOpType.mult)
                    nc.vector.tensor_tensor(out=shift[:, q:q + 1],
                                            in0=gb[("b", l)][:, q:q + 1], in1=musq[:],
                                            op=mybir.AluOpType.subtract)
                # ---- BN apply + ReLU -> hT (f16), then rows + AllGather ----
                for q in range(nfc):
                    nc.scalar.activation(
                        out=hT[q][:, 0:nown], in_=preBN[q][:, 0:nown],
                        func=mybir.ActivationFunctionType.Relu,
                        bias=shift[:, q:q + 1], scale=scale[:, q:q + 1],
                    )
                if l == 3:
                    return  # h3 is only consumed locally (layer 4 root term)
                for b2 in range(0, nblk, 2):
                    bl2 = [b for b in (b2, b2 + 1) if b < nblk]
                    w2 = len(bl2) * hid
                    tpr = psB.tile([P, 1024], f16, tag="tp")
                    for bi, b in enumerate(bl2):
                        for q in range(nfc):
                            nc.tensor.matmul(
                                out=tpr[:, bi * hid + q * P:bi * hid + (q + 1) * P],
                                lhsT=hT[q][:, b * P:(b + 1) * P],
                                rhs=ident[:], is_transpose=True)
                    rows = rp.tile([P, 1024], TAB_DT[l], tag="rows")
                    if (b2 // 2) % 2 == 0:
                        nc.vector.tensor_copy(out=rows[:, :w2], in_=tpr[:, :w2])
                    else:
                        nc.scalar.activation(
                            out=rows[:, :w2], in_=tpr[:, :w2],
                            func=mybir.ActivationFunctionType.Copy)
                    for bi, b in enumerate(bl2):
                        ns, ne = b * P, min((b + 1) * P, nown)
                        nc.sync.dma_start(
                            out=h_own[l][ns:ne, :],
                            in_=rows[:ne - ns, bi * hid:bi * hid + hid])
                nc.gpsimd.collective_compute(
                    "AllGather", mybir.AluOpType.bypass, replica_groups=rg,
                    ins=[h_own[l][:, :]], outs=[h_all[l][:, :]],
                )
                # precompute next layer's root term (Wr_{l+1}^T @ hT) into
                # preBN; runs on PE while the AllGather moves bytes
                nl = l + 1
                for nt2 in range(ntile):
                    ns2, ne2 = nt2 * 512, min((nt2 + 1) * 512, nown)
                    nn2 = ne2 - ns2
                    for fo in range(nfc):
                        rps = psC.tile([P, 512], f32, tag="dense")
                        for q in range(nfc):
                            nc.tensor.matmul(
                                out=rps[:, :nn2],
                                lhsT=W[("r", nl, q)][:, fo * P:(fo + 1) * P],
                                rhs=hT[q][:, ns2:ne2],
                                start=(q == 0), stop=(q == nfc - 1))
                        if (nt2 + fo) % 2 == 0:
                            nc.vector.tensor_copy(out=preBN[fo][:, ns2:ne2],
                                                  in_=rps[:, :nn2])
                        else:
                            nc.scalar.activation(
                                out=preBN[fo][:, ns2:ne2], in_=rps[:, :nn2],
                                func=mybir.ActivationFunctionType.Copy)

            # ================= layer 1 =================
            # precompute the L1 root term (x @ Wr1) into preBN on the idle PE
            # while the first gathers stream in
            for nt0 in range(ntile):
                ns0, ne0 = nt0 * 512, min((nt0 + 1) * 512, nown)
                nn0 = ne0 - ns0
                xt0 = sm.tile([in_f, 512], f16, tag="xTt", name="xTt")
                nc.sync.dma_start(out=xt0[:, :nn0], in_=xT[:, ns0:ne0])
                for fo in range(nfc):
                    rps0 = psC.tile([P, 512], f32, tag="dense")
                    nc.tensor.matmul(out=rps0[:, :nn0],
                                     lhsT=W[("r", 1, 0)][:, fo * P:(fo + 1) * P],
                                     rhs=xt0[:, :nn0],
                                     start=True, stop=True)
                    nc.vector.tensor_copy(out=preBN[fo][:, ns0:ne0],
                                          in_=rps0[:, :nn0])
            layer_123(1, x16[:, :], x16[BASE2:, :], 1, None, in_f, 128, f16,
                      root_pre=True)
            # ================= layers 2,3 =================
            for l in (2, 3):
                layer_123(l, h_all[l - 1][:, :], h_all[l - 1][BASE2:, :], nfc,
                          None, hid, hid, TAB_DT[l - 1], root_pre=True)
            # ================= layer 4 =================
            # y = h3 @ Wl4 (transposed), to rows, allgather
            for nt in range(ntile):
                ns, ne = nt * 512, min((nt + 1) * 512, nown)
                nn = ne - ns
                yps = psC.tile([P, 512], f32, tag="dense")
                for q in range(nfc):
                    nc.tensor.matmul(out=yps[:out_f, :nn],
                                     lhsT=W[("l", 4, q)][:, :out_f],
                                     rhs=hT[q][:, ns:ne],
                                     start=(q == 0), stop=(q == nfc - 1))
                ysb = sb.tile([P, 512], f16, tag="ysb")
                nc.vector.tensor_copy(out=ysb[:out_f, :nn], in_=yps[:out_f, :nn])
                for bi in range((nn + P - 1) // P):
                    b0 = bi * P
                    b1 = min(b0 + P, nn)
                    tpy = psB.tile([P, 512], f16, tag="tp")
                    nc.tensor.matmul(out=tpy[:b1 - b0, :out_f],
                                     lhsT=ysb[:out_f, b0:b1],
                                     rhs=ident[:out_f, :out_f],
                                     is_transpose=True)
                    yr = rp.tile([P, 128], f16, tag="yrows")
                    nc.vector.memset(yr[:], 0.0)
                    nc.vector.tensor_copy(out=yr[:b1 - b0, :out_f],
                                          in_=tpy[:b1 - b0, :out_f])
                    nc.sync.dma_start(out=y_own[ns + b0:ns + b1, :],
                                      in_=yr[:b1 - b0, :])
            for nt in range(ntile):
                ns, ne = nt * 512, min((nt + 1) * 512, nown)
                nn = ne - ns
                rps4 = psC.tile([P, 512], f32, tag="dense")
                for q in range(nfc):
                    nc.tensor.matmul(out=rps4[:out_f, :nn],
                                     lhsT=W[("r", 4, q)][:, :out_f],
                                     rhs=hT[q][:, ns:ne],
                                     start=(q == 0), stop=(q == nfc - 1))
                if nt % 2 == 0:
                    nc.vector.tensor_copy(out=preBN[0][:out_f, ns:ne],
                                          in_=rps4[:out_f, :nn])
                else:
                    nc.scalar.activation(out=preBN[0][:out_f, ns:ne],
                                         in_=rps4[:out_f, :nn],
                                         func=mybir.ActivationFunctionType.Copy)
            nc.gpsimd.collective_compute(
                "AllGather", mybir.AluOpType.bypass, replica_groups=rg,
                ins=[y_own[:, :]], outs=[y_all[:, :]],
            )
            # final: out = mean-agg(y) + h3 @ Wr4 + bl4
            for nt in range(ntile):
                ns, ne = nt * 512, min((nt + 1) * 512, nown)
                nn = ne - ns
                agg4T = sb.tile([P, 512], f16, tag="agg4T")
                pair_tiles4 = []
                for pr in (2 * nt, 2 * nt + 1):
                    if pr * 2 < nblk:
                        pair_tiles4 += aggregate_pair(pr, y_all[:, :],
                                                      y_all[BASE2:, :],
                                                      out_f, "4", 128, f16)
                for bi, b in enumerate(range(nt * 4, min(nt * 4 + 4, nblk))):
                    asb = pair_tiles4[bi]
                    tp = psB.tile([P, 512], f16, tag="tp")
                    nc.tensor.matmul(out=tp[:out_f, bi * P:(bi + 1) * P],
                                     lhsT=asb[:], rhs=ident[:], is_transpose=True)
                    nc.vector.tensor_copy(out=agg4T[:out_f, bi * P:(bi + 1) * P],
                                          in_=tp[:out_f, bi * P:(bi + 1) * P])
                osb = sb.tile([P, 512], f32, tag="osb")
                nc.vector.tensor_tensor(out=osb[:out_f, :nn],
                                        in0=preBN[0][:out_f, ns:ne],
                                        in1=agg4T[:out_f, :nn],
                                        op=mybir.AluOpType.add)
                nc.vector.tensor_scalar(out=osb[:out_f, :nn], in0=osb[:out_f, :nn],
                                        scalar1=bl4_t[:out_f, 0:1], scalar2=None,
                                        op0=mybir.AluOpType.add)
                for bi in range((nn + P - 1) // P):
                    b0, b1 = bi * P, min(bi * P + P, nn)
                    tpo = psB.tile([P, 512], f32, tag="tp")
                    nc.tensor.matmul(out=tpo[:b1 - b0, :out_f],
                                     lhsT=osb[:out_f, b0:b1],
                                     rhs=ident32[:out_f, :out_f],
                                     is_transpose=True)
                    orow = sb.tile([P, out_f], f32, tag="orow")
                    nc.vector.tensor_copy(out=orow[:b1 - b0, :],
                                          in_=tpo[:b1 - b0, :out_f])
                    nc.sync.dma_start(out=out_d[ns + b0:ns + b1, :],
                                      in_=orow[:b1 - b0, :])
    return nc


# chunk offsets per block, filled by build_inputs (shared plan state)
plan_choff = []


def _prep(plan):
    """Fill global chunk-offset table for the builder."""
    global plan_choff
    plan_choff = []
    off = 0
    for groups in plan.calls:
        plan_choff.append(off)
        off += sum(sum(ks) for _, ks in groups)


def _execute(nc, in_maps):
    from concourse.bass_utils import run_bass_kernel_spmd
    res = run_bass_kernel_spmd(nc, in_maps, list(range(NCORES)))
    return [res.results[c] for c in range(NCORES)]


def kernel(**inputs):
    x = np.asarray(inputs["x"], np.float32)
    edge_index = np.asarray(inputs["edge_index"])
    n_nodes, in_f = x.shape
    hid = inputs["Wl2"].shape[0]
    out_f = inputs["Wl4"].shape[1]
    nown = n_nodes // NCORES

    src = np.asarray(edge_index[0]).astype(np.int64)
    dst = np.asarray(edge_index[1]).astype(np.int64)
    deg = np.bincount(dst, minlength=n_nodes).astype(np.float32)
    deginv = (1.0 / np.maximum(deg, 1.0)).astype(np.float32)

    plans = _plan_all(n_nodes, edge_index)
    # pad chunk counts to the max across cores so one program fits all
    plans = _pad_plans(plans)
    _prep(plans[0])

    import time as _time
    _t0 = _time.perf_counter()
    nc = build_program(n_nodes, in_f, hid, out_f, plans[0])
    print(f"[kernel] program built in {_time.perf_counter() - _t0:.1f}s", flush=True)
    _t0 = _time.perf_counter()
    nc.compile()
    print(f"[kernel] bacc compile in {_time.perf_counter() - _t0:.1f}s", flush=True)

    x16 = np.zeros((n_nodes, 128), np.float16)
    x16[:, :in_f] = x.astype(np.float16)
    nblk = plans[0].nblk
    pad_n = nblk * P

    in_maps = []
    for c, p in enumerate(plans):
        xTc = np.zeros((in_f, pad_n), np.float16)
        xTc[:, :nown] = x[c * nown:(c + 1) * nown].T.astype(np.float16)
        dg = np.zeros(pad_n, np.float32)
        dg[:nown] = deginv[c * nown:(c + 1) * nown]
        im = {
            "x16": x16, "xT": xTc,
            "idx16": p.idx16 if p.idx16.size else np.zeros((P, 8), np.int16),
            "dst16": p.dst16 if p.dst16.size else np.zeros((P, 1), np.int16),
            "deginv": dg,
            "bl4": np.asarray(inputs["bl4"], np.float32),
        }
        for l in (1, 2, 3, 4):
            im[f"Wl{l}"] = np.asarray(inputs[f"Wl{l}"], np.float16)
            im[f"Wr{l}"] = np.asarray(inputs[f"Wr{l}"], np.float16)
        for l in (1, 2, 3):
            im[f"g{l}"] = np.asarray(inputs[f"g{l}"], np.float32)
            im[f"b{l}"] = np.asarray(inputs[f"b{l}"], np.float32)
        in_maps.append(im)

    global LAST_BUILD
    LAST_BUILD = (nc, in_maps)
    results = _execute(nc, in_maps)
    out = np.concatenate([results[c]["out"] for c in range(NCORES)], axis=0)
    return out.astype(np.float32)


_f8np = mybir.dt.np(fp8)


def _build_s(p):
    """One-hot S chunks [128, totch, 128] f16 from the dst16 table."""
    if p.dst16.size == 0 or p.totch == 0:
        return np.zeros((P, 1, P), np.float16)
    S = np.zeros((P, p.totch, P), np.float16)
    e = np.arange(P)
    for ch in range(p.totch):
        d = p.dst16[:, ch]
        m = d >= 0
        S[e[m], ch, d[m]] = 1.0
    return S


def _pad_plans(plans):
    """Pad every core's per-(block,group) chunk count to the cross-core max
    and rebuild idx16/dst16 accordingly, so one program serves all cores."""
    npair = plans[0].npair
    kmax = {}
    for pr in range(npair):
        for gi in range(2):
            nb = len(plans[0].calls[pr][gi][1])
            kmax[(pr, gi)] = [max(p.calls[pr][gi][1][i] for p in plans)
                              for i in range(nb)]
    for p in plans:
        idx_vals, dst_vals, calls = [], [], []
        off = 0
        orig_iv = _unwrap_idx(p.idx16, p.totch)
        for pr in range(npair):
            groups = []
            for gi in range(2):
                base_id, ks = p.calls[pr][gi]
                kms = kmax[(pr, gi)]
                for i, (k, km) in enumerate(zip(ks, kms)):
                    iv = np.zeros(km * P, np.int16)
                    dv = np.full((P, km), -1, np.int16)
                    if k:
                        iv[:k * P] = orig_iv[off * P:(off + k) * P]
                        dv[:, :k] = p.dst16[:, off:off + k]
                    off += k
                    idx_vals.append(iv)
                    dst_vals.append(dv)
                groups.append((base_id, list(kms)))
            calls.append(groups)
        p.calls = calls
        p.totch = sum(sum(kmax[(pr, gi)]) for pr in range(npair) for gi in range(2))
        iv = np.concatenate(idx_vals) if idx_vals else np.zeros(0, np.int16)
        w = iv.reshape(-1, 16).T if iv.size else np.zeros((16, 0), np.int16)
        p.idx16 = np.tile(w, (8, 1)).copy()
        p.dst16 = np.concatenate(dst_vals, axis=1).copy() if dst_vals else \
            np.zeros((P, 1), np.int16)
    return plans


def _unwrap_idx(idx16, totch):
    """Inverse of the 16-partition wrap: [128, totch*8] -> flat [totch*128]."""
    if idx16.size == 0:
        return np.zeros(0, np.int16)
    return idx16[:16, :].T.reshape(-1)


# revision 42
# speedup vs baseline: 1.0765x; 1.0261x over previous
"""DeepGraphSAGE (4x SAGEConv + BN/ReLU) on 8 Trainium2 NeuronCores.

Sharding: nodes partitioned across 8 cores (6250 dst nodes each). Each layer:
  - mean-aggregate neighbor features via dma_gather (rows of the allgathered
    H table) + one-hot selection matmuls accumulating in PSUM. The one-hot
    S matrices are built on-chip (iota + is_equal) from an int16 dst table.
  - dense transforms computed in transposed layout (features on partitions)
  - BatchNorm stats via bn_stats/bn_aggr + tiny cross-core AllReduce
  - PE transposes back to row layout, AllGather of H for the next layer.
Hidden-state tables (h1, h2) travel as fp8e3 (E3M4) on the wire and in the
gather table; weights/root terms stay fp16; accumulation/stats are fp32.
"""
import sys
import numpy as np

for p in ("/opt/trn_rl_repo",):
    if p not in sys.path:
        sys.path.append(p)

import concourse.bass as bass
import concourse.bacc as bacc
import concourse.mybir as mybir
from concourse.tile import TileContext
from concourse.masks import make_identity

f32 = mybir.dt.float32
f16 = mybir.dt.float16
fp8 = mybir.dt.float8e3
i16 = mybir.dt.int16

NCORES = 8
P = 128
SPLIT = 32768          # int16 index limit
BASE2 = 17232          # second gather base (recomputed per problem size)
EPS = 1e-5
LAST_BUILD = None
# wire/table dtype of the allgathered hidden state per layer boundary
TAB_DT = {1: fp8, 2: fp8}


# ---------------------------------------------------------------- host prep
class Plan:
    """Per-core gather/selection plan derived from edge_index."""

    def __init__(self, n_nodes, src, dst, core):
        self.n_own = n_nodes // NCORES
        self.nblk = (self.n_own + P - 1) // P
        lo = core * self.n_own
        m = (dst >= lo) & (dst < lo + self.n_own)
        es = src[m].astype(np.int64)
        ed = (dst[m] - lo).astype(np.int64)
        order = np.argsort(ed, kind="stable")
        es, ed = es[order], ed[order]
        bounds = np.searchsorted(ed, np.arange(0, self.nblk + 1) * P)

        idx_vals = []     # flat int16 index stream (multiple of 128 per group)
        dst_vals = []     # per chunk: [128] i16 dst-in-block (-1 pad)
        calls = []        # per PAIR: [(base_id, [k per block in pair]), ...]
        npair = (self.nblk + 1) // 2
        for pr in range(npair):
            blocks = [b for b in (2 * pr, 2 * pr + 1) if b < self.nblk]
            groups = []
            for base_id in (0, 1):
                ks = []
                for b in blocks:
                    e0, e1 = bounds[b], bounds[b + 1]
                    bs, bd = es[e0:e1], ed[e0:e1] - b * P
                    msel = (bs < SPLIT) if base_id == 0 else (bs >= SPLIT)
                    gs, gd = bs[msel], bd[msel]
                    k = (len(gs) + P - 1) // P
                    ks.append(k)
                    if k == 0:
                        continue
                    padded = np.zeros(k * P, np.int64)
                    padded[: len(gs)] = gs - (BASE2 if base_id else 0)
                    idx_vals.append(padded.astype(np.int16))
                    dpad = np.full(k * P, -1, np.int64)
                    dpad[: len(gd)] = gd
                    for j in range(k):
                        dst_vals.append(dpad[j * P:(j + 1) * P].astype(np.int16))
                groups.append((base_id, ks))
            calls.append(groups)

        self.calls = calls
        self.npair = npair
        self.totch = len(dst_vals)
        iv = np.concatenate(idx_vals) if idx_vals else np.zeros(0, np.int16)
        # dma_gather index layout: position i -> [i%16, i//16], replicated 8x
        w = iv.reshape(-1, 16).T if iv.size else np.zeros((16, 0), np.int16)
        self.idx16 = np.tile(w, (8, 1)).copy()           # [128, totch*8] i16
        self.dst16 = np.stack(dst_vals, axis=1).copy() if dst_vals else \
            np.zeros((P, 0), np.int16)                   # [128, totch] i16


def _plan_all(n_nodes, edge_index):
    global BASE2
    BASE2 = max(0, n_nodes - SPLIT)
    src = np.asarray(edge_index[0])
    dst = np.asarray(edge_index[1])
    return [Plan(n_nodes, src, dst, c) for c in range(NCORES)]


# ---------------------------------------------------------------- program
def build_program(n_nodes, in_f, hid, out_f, plan0):
    """One SPMD program (same for all cores; per-core data differs)."""
    nown = plan0.n_own
    nblk = plan0.nblk
    pad_n = nblk * P
    ntile = (nown + 511) // 512
    nfc = hid // P               # 4 feature chunks of the hidden dim
    totch = plan0.totch
    calls = plan0.calls

    nc = bacc.Bacc("TRN2", target_bir_lowering=False, debug=False,
                   num_devices=NCORES, num_swdge_queues=4)

    # ---- I/O ----
    x16 = nc.dram_tensor("x16", [n_nodes, 128], f16, kind="ExternalInput")
    xT = nc.dram_tensor("xT", [in_f, pad_n], f16, kind="ExternalInput")
    idx16_d = nc.dram_tensor("idx16", [P, max(totch * 8, 8)], i16, kind="ExternalInput")
    dst16_d = nc.dram_tensor("dst16", [P, max(totch, 1)], i16, kind="ExternalInput")
    deginv_d = nc.dram_tensor("deginv", [pad_n], f32, kind="ExternalInput")
    wl_d, wr_d, g_d, b_d = {}, {}, {}, {}
    dims = [(in_f, hid), (hid, hid), (hid, hid), (hid, out_f)]
    for l, (fi, fo) in enumerate(dims, start=1):
        wl_d[l] = nc.dram_tensor(f"Wl{l}", [fi, fo], f16, kind="ExternalInput")
        wr_d[l] = nc.dram_tensor(f"Wr{l}", [fi, fo], f16, kind="ExternalInput")
    for l in (1, 2, 3):
        g_d[l] = nc.dram_tensor(f"g{l}", [hid], f32, kind="ExternalInput")
        b_d[l] = nc.dram_tensor(f"b{l}", [hid], f32, kind="ExternalInput")
    bl4_d = nc.dram_tensor("bl4", [out_f], f32, kind="ExternalInput")
    out_d = nc.dram_tensor("out", [nown, out_f], f32, kind="ExternalOutput")

    # ---- internal DRAM ----
    h_own = {l: nc.dram_tensor(f"h{l}_own", [nown, hid], TAB_DT[l]) for l in (1, 2)}
    h_all = {l: nc.dram_tensor(f"h{l}_all", [n_nodes, hid], TAB_DT[l], addr_space="Shared")
             for l in (1, 2)}
    y_own = nc.dram_tensor("y_own", [nown, 128], f16)
    y_all = nc.dram_tensor("y_all", [n_nodes, 128], f16, addr_space="Shared")
    st_in = {l: nc.dram_tensor(f"st{l}_in", [P, 8], f32) for l in (1, 2, 3)}
    st_out = {l: nc.dram_tensor(f"st{l}_out", [P, 8], f32, addr_space="Shared")
              for l in (1, 2, 3)}
    rg = [list(range(NCORES))]

    with TileContext(nc) as tc:
        with (
            tc.tile_pool(name="const", bufs=1) as cp,
            tc.tile_pool(name="sbuf", bufs=2) as sb,
            tc.tile_pool(name="small", bufs=3) as sm,
            tc.tile_pool(name="spool", bufs=2) as sp,
            tc.tile_pool(name="gpool", bufs=3) as gp,
            tc.tile_pool(name="rows", bufs=3) as rp,
            tc.tile_pool(name="psA", bufs=2, space="PSUM") as psA,
            tc.tile_pool(name="psB", bufs=2, space="PSUM") as psB,
            tc.tile_pool(name="psC", bufs=2, space="PSUM") as psC,
        ):
            ident = cp.tile([P, P], f16)
            make_identity(nc, ident[:])
            ident32 = cp.tile([P, P], f32)
            make_identity(nc, ident32[:])
            iota_t = cp.tile([P, P], i16)
            nc.gpsimd.iota(iota_t[:], pattern=[[1, P]], base=0,
                           channel_multiplier=0,
                           allow_small_or_imprecise_dtypes=True)
            deginv_t = cp.tile([P, nblk], f32)
            nc.sync.dma_start(out=deginv_t[:],
                              in_=deginv_d[:].rearrange("(b p) -> p b", p=P))
            # gather indices + dst-in-block tables resident in SBUF
            idxc = cp.tile([P, max(totch * 8, 8)], i16)
            nc.sync.dma_start(out=idxc[:], in_=idx16_d[:, :])
            dstc = cp.tile([P, max(totch, 1)], i16)
            nc.sync.dma_start(out=dstc[:], in_=dst16_d[:, :])
            # weights resident in SBUF, per fi-chunk tiles
            W = {}
            for l, (fi, fo) in enumerate(dims, start=1):
                kc = (fi + P - 1) // P
                for (nm, dram) in (("l", wl_d[l]), ("r", wr_d[l])):
                    for q in range(kc):
                        r0, r1 = q * P, min((q + 1) * P, fi)
                        t = cp.tile([r1 - r0, fo], f16, tag=f"W{nm}{l}_{q}")
                        nc.sync.dma_start(out=t[:], in_=dram[r0:r1, :])
                        W[(nm, l, q)] = t
            gb = {}
            for l in (1, 2, 3):
                for nm, dram in (("g", g_d[l]), ("b", b_d[l])):
                    t = cp.tile([P, nfc], f32, tag=f"{nm}{l}")
                    nc.sync.dma_start(out=t[:], in_=dram[:].rearrange("(c p) -> p c", p=P))
                    gb[(nm, l)] = t
            bl4_t = cp.tile([P, 1], f32)
            nc.sync.dma_start(out=bl4_t[:out_f, :], in_=bl4_d[:, None])

            # persistent hidden state (transposed) + pre-BN buffer
            hT = [cp.tile([P, pad_n], f16, tag=f"hT{q}", name=f"hT{q}") for q in range(nfc)]
            preBN = [cp.tile([P, pad_n], f16, tag=f"preBN{q}", name=f"preBN{q}") for q in range(nfc)]
            if pad_n > nown:
                for q in range(nfc):
                    nc.vector.memset(hT[q][:, nown:pad_n], 0.0)

            gq = [0]  # gather queue round-robin state

            def aggregate_pair(pr, src_table, src_table2, width, tagsfx,
                               row_elems, dt):
                """Mean-aggregate both blocks of pair pr. One dma_gather per
                base-group spanning the pair. Returns list of f16 tiles."""
                groups = calls[pr]
                blocks = [b for b in (2 * pr, 2 * pr + 1) if b < nblk]
                ktot = sum(sum(ks) for _, ks in groups)
                out_tiles = []
                if ktot == 0:
                    for bi in range(len(blocks)):
                        z = sm.tile([P, width], f16, tag=f"agg{tagsfx}{bi}",
                                    name=f"aggz{bi}")
                        nc.vector.memset(z[:], 0.0)
                        out_tiles.append(z)
                    return out_tiles
                ch0 = plan_choff[pr]
                stile = sp.tile([P, ktot, P], dt, tag="S")
                nc.vector.tensor_tensor(
                    out=stile[:],
                    in0=dstc[:, ch0:ch0 + ktot].unsqueeze(2).broadcast_to([P, ktot, P]),
                    in1=iota_t[:].unsqueeze(1).broadcast_to([P, ktot, P]),
                    op=mybir.AluOpType.is_equal,
                )
                g = gp.tile([P, ktot, row_elems], dt, tag="G")
                koff = 0
                for base_id, ks in groups:
                    k = sum(ks)
                    if k == 0:
                        continue
                    src_ap = src_table if base_id == 0 else src_table2
                    nc.gpsimd.dma_gather(
                        out_ap=g[:, koff:koff + k, :],
                        in_ap=src_ap,
                        idxs_ap=idxc[:, (ch0 + koff) * 8:(ch0 + koff + k) * 8],
                        num_idxs=k * P, num_idxs_reg=k * P,
                        elem_size=row_elems, single_packet=False,
                        queue_num=gq[0] % 4,
                    )
                    gq[0] += 1
                    koff += k
                # per-block PSUM accumulation over that block's chunks
                for bi, b in enumerate(blocks):
                    agg_ps = psA.tile([P, 512], f32, tag=f"agg_ps{bi}",
                                      name=f"agg_ps{bi}")
                    mm_idx = []
                    koff = 0
                    for base_id, ks in groups:
                        pre = 0
                        for i2, k2 in enumerate(ks):
                            if i2 == bi:
                                mm_idx += list(range(koff + pre, koff + pre + k2))
                            pre += k2
                        koff += sum(ks)
                    if not mm_idx:
                        z = sm.tile([P, width], f16, tag=f"agg{tagsfx}{bi}",
                                    name=f"aggz2{bi}")
                        nc.vector.memset(z[:], 0.0)
                        out_tiles.append(z)
                        continue
                    for n_, j in enumerate(mm_idx):
                        nc.tensor.matmul(
                            out=agg_ps[:, :width],
                            lhsT=stile[:, j, :], rhs=g[:, j, :width],
                            start=(n_ == 0), stop=(n_ == len(mm_idx) - 1),
                        )
                    asb = sm.tile([P, width], f16, tag=f"agg{tagsfx}{bi}",
                                  name=f"asb{bi}")
                    nc.vector.tensor_scalar(
                        out=asb[:], in0=agg_ps[:, :width],
                        scalar1=deginv_t[:, b:b + 1], scalar2=None,
                        op0=mybir.AluOpType.mult,
                    )
                    out_tiles.append(asb)
                return out_tiles

            def layer_123(l, src_rows, src_rows2, fi_chunks, rhs_for_fi, width,
                          row_elems, dt, root_pre=False):
                """One SAGE layer with BN+ReLU. rhs_for_fi(q, ns, ne) gives the
                [K, n] rhs AP of the root term for fi-chunk q; aggregation uses
                src_rows tables at `width` features."""
                sums = [sb.tile([P, ntile], f32, tag=f"sums{q}", name=f"sums{q}")
                        for q in range(nfc)]
                sumsqs = [sb.tile([P, ntile], f32, tag=f"sumsq{q}", name=f"sumsq{q}")
                          for q in range(nfc)]
                for nt in range(ntile):
                    ns, ne = nt * 512, min((nt + 1) * 512, nown)
                    nn = ne - ns
                    # aggregate the (up to) 4 dst blocks of this node tile
                    aggT = (sb.tile([in_f, 512], f16, tag="aggT", name="aggT")
                            if width == in_f else None)
                    aggTq = ([sb.tile([P, 512], f16, tag=f"aggT{q}", name=f"aggT{q}")
                              for q in range(fi_chunks)] if width > in_f else None)
                    pair_tiles = []
                    for pr in (2 * nt, 2 * nt + 1):
                        if pr * 2 < nblk:
                            pair_tiles += aggregate_pair(pr, src_rows, src_rows2,
                                                         width, "sb", row_elems, dt)
                    for bi, b in enumerate(range(nt * 4, min(nt * 4 + 4, nblk))):
                        asb = pair_tiles[bi]
                        tp = psB.tile([P, 512], f16, tag="tp")
                        if width == in_f:
                            nc.tensor.matmul(out=tp[:width, bi * P:(bi + 1) * P],
                                             lhsT=asb[:], rhs=ident[:],
                                             is_transpose=True)
                            nc.vector.tensor_copy(out=aggT[:width, bi * P:(bi + 1) * P],
                                                  in_=tp[:width, bi * P:(bi + 1) * P])
                        else:
                            for q in range(fi_chunks):
                                nc.tensor.matmul(out=tp[:, q * P:(q + 1) * P],
                                                 lhsT=asb[:, q * P:(q + 1) * P],
                                                 rhs=ident[:], is_transpose=True)
                                nc.vector.tensor_copy(out=aggTq[q][:, bi * P:(bi + 1) * P],
                                                      in_=tp[:, q * P:(q + 1) * P])
                    # dense: out^T [fo chunk, nodes]
                    for fo in range(nfc):
                        dps = psC.tile([P, 512], f32, tag="dense")
                        nmm = fi_chunks if root_pre else 2 * fi_chunks
                        mm = 0
                        for q in range(fi_chunks):
                            rhs_agg = (aggT[:width, :nn] if width == in_f
                                       else aggTq[q][:, :nn])
                            nc.tensor.matmul(out=dps[:, :nn],
                                             lhsT=W[("l", l, q)][:, fo * P:(fo + 1) * P],
                                             rhs=rhs_agg, start=(mm == 0),
                                             stop=(mm == nmm - 1))
                            mm += 1
                            if not root_pre:
                                nc.tensor.matmul(out=dps[:, :nn],
                                                 lhsT=W[("r", l, q)][:, fo * P:(fo + 1) * P],
                                                 rhs=rhs_for_fi(q, ns, ne),
                                                 start=False, stop=(mm == nmm - 1))
                                mm += 1
                        if root_pre:
                            # preBN holds the precomputed root term; add agg.
                            nc.vector.scalar_tensor_tensor(
                                out=preBN[fo][:, ns:ne], in0=dps[:, :nn],
                                scalar=1.0, in1=preBN[fo][:, ns:ne],
                                op0=mybir.AluOpType.mult,
                                op1=mybir.AluOpType.add,
                                accum_out=sums[fo][:, nt:nt + 1])
                            scr = sm.tile([P, 512], f16, tag="scr", name="scr")
                            nc.scalar.activation(
                                out=scr[:, :nn], in_=preBN[fo][:, ns:ne],
                                func=mybir.ActivationFunctionType.Square,
                                accum_out=sumsqs[fo][:, nt:nt + 1])
                        else:
                            scr = sm.tile([P, 512], f16, tag="scr", name="scr")
                            nc.scalar.activation(
                                out=scr[:, :nn], in_=dps[:, :nn],
                                func=mybir.ActivationFunctionType.Square,
                                accum_out=sumsqs[fo][:, nt:nt + 1])
                            nc.vector.tensor_scalar(
                                out=preBN[fo][:, ns:ne], in0=dps[:, :nn],
                                scalar1=1.0, scalar2=None,
                                op0=mybir.AluOpType.mult,
                                op1=mybir.AluOpType.add,
                                accum_out=sums[fo][:, nt:nt + 1])
                # ---- BN statistics + cross-core allreduce ----
                # pack = per-core [S1, S2] per feature; AllReduce sums them.
                pack = sb.tile([P, 8], f32, tag="pack")
                for q in range(nfc):
                    nc.vector.reduce_sum(out=pack[:, 2 * q:2 * q + 1],
                                         in_=sums[q][:], axis=mybir.AxisListType.X)
                    nc.vector.reduce_sum(out=pack[:, 2 * q + 1:2 * q + 2],
                                         in_=sumsqs[q][:], axis=mybir.AxisListType.X)
                nc.sync.dma_start(out=st_in[l][:, :], in_=pack[:])
                nc.gpsimd.collective_compute(
                    "AllReduce", mybir.AluOpType.add, replica_groups=rg,
                    ins=[st_in[l][:, :]], outs=[st_out[l][:, :]],
                )
                red = sb.tile([P, 8], f32, tag="red")
                nc.sync.dma_start(out=red[:], in_=st_out[l][:, :])
                scale = sb.tile([P, nfc], f32, tag="scale")
                shift = sb.tile([P, nfc], f32, tag="shift")
                inv_n = 1.0 / float(n_nodes)
                for q in range(nfc):
                    mu = sb.tile([P, 1], f32, tag="mu")
                    var = sb.tile([P, 1], f32, tag="var")
                    nc.vector.tensor_scalar(out=mu[:], in0=red[:, 2 * q:2 * q + 1],
                                            scalar1=inv_n, scalar2=None,
                                            op0=mybir.AluOpType.mult)
                    nc.vector.tensor_scalar(out=var[:], in0=red[:, 2 * q + 1:2 * q + 2],
                                            scalar1=inv_n, scalar2=None,
                                            op0=mybir.AluOpType.mult)
                    musq = sb.tile([P, 1], f32, tag="musq")
                    nc.vector.tensor_tensor(out=musq[:], in0=mu[:], in1=mu[:],
                                            op=mybir.AluOpType.mult)
                    nc.vector.tensor_tensor(out=var[:], in0=var[:], in1=musq[:],
                                            op=mybir.AluOpType.subtract)
                    nc.vector.tensor_scalar(out=var[:], in0=var[:], scalar1=EPS,
                                            scalar2=None, op0=mybir.AluOpType.add)
                    nc.vector.reciprocal(out=var[:], in_=var[:])
                    rs = sb.tile([P, 1], f32, tag="rs")
                    nc.scalar.activation(out=rs[:], in_=var[:],
                                         func=mybir.ActivationFunctionType.Sqrt)
                    nc.vector.tensor_tensor(out=scale[:, q:q + 1], in0=rs[:],
                                            in1=gb[("g", l)][:, q:q + 1],
                                            op=mybir.AluOpType.mult)
                    nc.vector.tensor_tensor(out=musq[:], in0=mu[:],
                                            in1=scale[:, q:q + 1],
                                            op=mybir.AluOpType.mult)
                    nc.vector.tensor_tensor(out=shift[:, q:q + 1],
                                            in0=gb[("b", l)][:, q:q + 1], in1=musq[:],
                                            op=mybir.AluOpType.subtract)
                # ---- BN apply + ReLU -> hT (f16), then rows + AllGather ----
                for q in range(nfc):
                    nc.scalar.activation(
                        out=hT[q][:, 0:nown], in_=preBN[q][:, 0:nown],
                        func=mybir.ActivationFunctionType.Relu,
                        bias=shift[:, q:q + 1], scale=scale[:, q:q + 1],
                    )
                if l == 3:
                    return  # h3 is only consumed locally (layer 4 root term)
                for b2 in range(0, nblk, 2):
                    bl2 = [b for b in (b2, b2 + 1) if b < nblk]
                    w2 = len(bl2) * hid
                    tpr = psB.tile([P, 1024], f16, tag="tp")
                    for bi, b in enumerate(bl2):
                        for q in range(nfc):
                            nc.tensor.matmul(
                                out=tpr[:, bi * hid + q * P:bi * hid + (q + 1) * P],
                                lhsT=hT[q][:, b * P:(b + 1) * P],
                                rhs=ident[:], is_transpose=True)
                    rows = rp.tile([P, 1024], TAB_DT[l], tag="rows")
                    if (b2 // 2) % 2 == 0:
                        nc.vector.tensor_copy(out=rows[:, :w2], in_=tpr[:, :w2])
                    else:
                        nc.scalar.activation(
                            out=rows[:, :w2], in_=tpr[:, :w2],
                            func=mybir.ActivationFunctionType.Copy)
                    for bi, b in enumerate(bl2):
                        ns, ne = b * P, min((b + 1) * P, nown)
                        nc.sync.dma_start(
                            out=h_own[l][ns:ne, :],
                            in_=rows[:ne - ns, bi * hid:bi * hid + hid])
                nc.gpsimd.collective_compute(
                    "AllGather", mybir.AluOpType.bypass, replica_groups=rg,
                    ins=[h_own[l][:, :]], outs=[h_all[l][:, :]],
                )
                # precompute next layer's root term (Wr_{l+1}^T @ hT) into
                # preBN; runs on PE while the AllGather moves bytes
                nl = l + 1
                for nt2 in range(ntile):
                    ns2, ne2 = nt2 * 512, min((nt2 + 1) * 512, nown)
                    nn2 = ne2 - ns2
                    for fo in range(nfc):
                        rps = psC.tile([P, 512], f32, tag="dense")
                        for q in range(nfc):
                            nc.tensor.matmul(
                                out=rps[:, :nn2],
                                lhsT=W[("r", nl, q)][:, fo * P:(fo + 1) * P],
                                rhs=hT[q][:, ns2:ne2],
                                start=(q == 0), stop=(q == nfc - 1))
                        if (nt2 + fo) % 2 == 0:
                            nc.vector.tensor_copy(out=preBN[fo][:, ns2:ne2],
                                                  in_=rps[:, :nn2])
                        else:
                            nc.scalar.activation(
                                out=preBN[fo][:, ns2:ne2], in_=rps[:, :nn2],
                                func=mybir.ActivationFunctionType.Copy)

            # ================= layer 1 =================
            def xT_rhs(q, ns, ne):
                xt = sm.tile([in_f, 512], f16, tag="xTt", name="xTt")
                nc.sync.dma_start(out=xt[:, :ne - ns], in_=xT[:, ns:ne])
                return xt[:, :ne - ns]
            layer_123(1, x16[:, :], x16[BASE2:, :], 1, xT_rhs, in_f, 128, f16)
            # ================= layers 2,3 =================
            for l in (2, 3):
                layer_123(l, h_all[l - 1][:, :], h_all[l - 1][BASE2:, :], nfc,
                          None, hid, hid, TAB_DT[l - 1], root_pre=True)
            # ================= layer 4 =================
            # y = h3 @ Wl4 (transposed), to rows, allgather
            for nt in range(ntile):
                ns, ne = nt * 512, min((nt + 1) * 512, nown)
                nn = ne - ns
                yps = psC.tile([P, 512], f32, tag="dense")
                for q in range(nfc):
                    nc.tensor.matmul(out=yps[:out_f, :nn],
                                     lhsT=W[("l", 4, q)][:, :out_f],
                                     rhs=hT[q][:, ns:ne],
                                     start=(q == 0), stop=(q == nfc - 1))
                ysb = sb.tile([P, 512], f16, tag="ysb")
                nc.vector.tensor_copy(out=ysb[:out_f, :nn], in_=yps[:out_f, :nn])
                for bi in range((nn + P - 1) // P):
                    b0 = bi * P
                    b1 = min(b0 + P, nn)
                    tpy = psB.tile([P, 512], f16, tag="tp")
                    nc.tensor.matmul(out=tpy[:b1 - b0, :out_f],
                                     lhsT=ysb[:out_f, b0:b1],
                                     rhs=ident[:out_f, :out_f],
                                     is_transpose=True)
                    yr = sb.tile([P, 128], f16, tag="yrows")
                    nc.vector.memset(yr[:], 0.0)
                    nc.vector.tensor_copy(out=yr[:b1 - b0, :out_f],
                                          in_=tpy[:b1 - b0, :out_f])
                    nc.sync.dma_start(out=y_own[ns + b0:ns + b1, :],
                                      in_=yr[:b1 - b0, :])
            for nt in range(ntile):
                ns, ne = nt * 512, min((nt + 1) * 512, nown)
                nn = ne - ns
                rps4 = psC.tile([P, 512], f32, tag="dense")
                for q in range(nfc):
                    nc.tensor.matmul(out=rps4[:out_f, :nn],
                                     lhsT=W[("r", 4, q)][:, :out_f],
                                     rhs=hT[q][:, ns:ne],
                                     start=(q == 0), stop=(q == nfc - 1))
                if nt % 2 == 0:
                    nc.vector.tensor_copy(out=preBN[0][:out_f, ns:ne],
                                          in_=rps4[:out_f, :nn])
                else:
                    nc.scalar.activation(out=preBN[0][:out_f, ns:ne],
                                         in_=rps4[:out_f, :nn],
                                         func=mybir.ActivationFunctionType.Copy)
            nc.gpsimd.collective_compute(
                "AllGather", mybir.AluOpType.bypass, replica_groups=rg,
                ins=[y_own[:, :]], outs=[y_all[:, :]],
            )
            # final: out = mean-agg(y) + h3 @ Wr4 + bl4
            for nt in range(ntile):
                ns, ne = nt * 512, min((nt + 1) * 512, nown)
                nn = ne - ns
                agg4T = sb.tile([P, 512], f16, tag="agg4T")
                pair_tiles4 = []
                for pr in (2 * nt, 2 * nt + 1):
                    if pr * 2 < nblk:
                        pair_tiles4 += aggregate_pair(pr, y_all[:, :],
                                                      y_all[BASE2:, :],
                                                      out_f, "4", 128, f16)
                for bi, b in enumerate(range(nt * 4, min(nt * 4 + 4, nblk))):
                    asb = pair_tiles4[bi]
                    tp = psB.tile([P, 512], f16, tag="tp")
                    nc.tensor.matmul(out=tp[:out_f, bi * P:(bi + 1) * P],
                                     lhsT=asb[:], rhs=ident[:], is_transpose=True)
                    nc.vector.tensor_copy(out=agg4T[:out_f, bi * P:(bi + 1) * P],
                                          in_=tp[:out_f, bi * P:(bi + 1) * P])
                osb = sb.tile([P, 512], f32, tag="osb")
                nc.vector.tensor_tensor(out=osb[:out_f, :nn],
                                        in0=preBN[0][:out_f, ns:ne],
                                        in1=agg4T[:out_f, :nn],
                                        op=mybir.AluOpType.add)
                nc.vector.tensor_scalar(out=osb[:out_f, :nn], in0=osb[:out_f, :nn],
                                        scalar1=bl4_t[:out_f, 0:1], scalar2=None,
                                        op0=mybir.AluOpType.add)
                for bi in range((nn + P - 1) // P):
                    b0, b1 = bi * P, min(bi * P + P, nn)
                    tpo = psB.tile([P, 512], f32, tag="tp")
                    nc.tensor.matmul(out=tpo[:b1 - b0, :out_f],
                                     lhsT=osb[:out_f, b0:b1],
                                     rhs=ident32[:out_f, :out_f],
                                     is_transpose=True)
                    orow = sb.tile([P, out_f], f32, tag="orow")
                    nc.vector.tensor_copy(out=orow[:b1 - b0, :],
                                          in_=tpo[:b1 - b0, :out_f])
                    nc.sync.dma_start(out=out_d[ns + b0:ns + b1, :],
                                      in_=orow[:b1 - b0, :])
    return nc


# chunk offsets per block, filled by build_inputs (shared plan state)
plan_choff = []


def _prep(plan):
    """Fill global chunk-offset table for the builder."""
    global plan_choff
    plan_choff = []
    off = 0
    for groups in plan.calls:
        plan_choff.append(off)
        off += sum(sum(ks) for _, ks in groups)


def _execute(nc, in_maps):
    from concourse.bass_utils import run_bass_kernel_spmd
    res = run_bass_kernel_spmd(nc, in_maps, list(range(NCORES)))
    return [res.results[c] for c in range(NCORES)]


def _balance_perm(n_nodes, dst):
    """Permutation new->old assigning nodes to (core, block) bins so the
    per-block-position degree sums align across cores (kills most of the
    cross-core kmax padding in the gather plans). Snake-deal by degree."""
    nown = n_nodes // NCORES
    nblk = (nown + P - 1) // P
    last_cap = nown - (nblk - 1) * P
    caps = np.full((NCORES, nblk), P, np.int64)
    caps[:, nblk - 1] = last_cap
    deg = np.bincount(dst, minlength=n_nodes)
    order = np.argsort(-deg, kind="stable")
    bins = [[] for _ in range(NCORES * nblk)]
    flat_caps = caps.reshape(-1)
    active = list(range(NCORES * nblk))
    i = 0
    fwd = True
    pos = 0
    while i < n_nodes:
        seq = active if fwd else active[::-1]
        for b in seq:
            if i >= n_nodes:
                break
            bins[b].append(order[i])
            i += 1
        fwd = not fwd
        active = [b for b in active if len(bins[b]) < flat_caps[b]]
    perm = np.empty(n_nodes, np.int64)
    k = 0
    for c in range(NCORES):
        for b in range(nblk):
            members = bins[c * nblk + b]
            perm[k:k + len(members)] = members
            k += len(members)
    return perm


def kernel(**inputs):
    x = np.asarray(inputs["x"], np.float32)
    edge_index = np.asarray(inputs["edge_index"])
    # relabel nodes so block degree sums balance across cores; the device
    # program runs entirely in permuted space, output is unpermuted below
    _dst0 = np.asarray(edge_index[1]).astype(np.int64)
    perm = _balance_perm(x.shape[0], _dst0)
    inv = np.empty_like(perm)
    inv[perm] = np.arange(perm.size)
    x = x[perm]
    edge_index = inv[np.asarray(edge_index).astype(np.int64)]
    n_nodes, in_f = x.shape
    hid = inputs["Wl2"].shape[0]
    out_f = inputs["Wl4"].shape[1]
    nown = n_nodes // NCORES

    src = np.asarray(edge_index[0]).astype(np.int64)
    dst = np.asarray(edge_index[1]).astype(np.int64)
    deg = np.bincount(dst, minlength=n_nodes).astype(np.float32)
    deginv = (1.0 / np.maximum(deg, 1.0)).astype(np.float32)

    plans = _plan_all(n_nodes, edge_index)
    # pad chunk counts to the max across cores so one program fits all
    plans = _pad_plans(plans)
    _prep(plans[0])

    import time as _time
    _t0 = _time.perf_counter()
    nc = build_program(n_nodes, in_f, hid, out_f, plans[0])
    print(f"[kernel] program built in {_time.perf_counter() - _t0:.1f}s", flush=True)
    _t0 = _time.perf_counter()
    nc.compile()
    print(f"[kernel] bacc compile in {_time.perf_counter() - _t0:.1f}s", flush=True)

    x16 = np.zeros((n_nodes, 128), np.float16)
    x16[:, :in_f] = x.astype(np.float16)
    nblk = plans[0].nblk
    pad_n = nblk * P

    in_maps = []
    for c, p in enumerate(plans):
        xTc = np.zeros((in_f, pad_n), np.float16)
        xTc[:, :nown] = x[c * nown:(c + 1) * nown].T.astype(np.float16)
        dg = np.zeros(pad_n, np.float32)
        dg[:nown] = deginv[c * nown:(c + 1) * nown]
        im = {
            "x16": x16, "xT": xTc,
            "idx16": p.idx16 if p.idx16.size else np.zeros((P, 8), np.int16),
            "dst16": p.dst16 if p.dst16.size else np.zeros((P, 1), np.int16),
            "deginv": dg,
            "bl4": np.asarray(inputs["bl4"], np.float32),
        }
        for l in (1, 2, 3, 4):
            im[f"Wl{l}"] = np.asarray(inputs[f"Wl{l}"], np.float16)
            im[f"Wr{l}"] = np.asarray(inputs[f"Wr{l}"], np.float16)
        for l in (1, 2, 3):
            im[f"g{l}"] = np.asarray(inputs[f"g{l}"], np.float32)
            im[f"b{l}"] = np.asarray(inputs[f"b{l}"], np.float32)
        in_maps.append(im)

    global LAST_BUILD
    LAST_BUILD = (nc, in_maps)
    results = _execute(nc, in_maps)
    out = np.concatenate([results[c]["out"] for c in range(NCORES)], axis=0)
    out_full = np.empty_like(out)
    out_full[perm] = out
    return out_full.astype(np.float32)


_f8np = mybir.dt.np(fp8)


def _build_s(p):
    """One-hot S chunks [128, totch, 128] f16 from the dst16 table."""
    if p.dst16.size == 0 or p.totch == 0:
        return np.zeros((P, 1, P), np.float16)
    S = np.zeros((P, p.totch, P), np.float16)
    e = np.arange(P)
    for ch in range(p.totch):
        d = p.dst16[:, ch]
        m = d >= 0
        S[e[m], ch, d[m]] = 1.0
    return S


def _pad_plans(plans):
    """Pad every core's per-(block,group) chunk count to the cross-core max
    and rebuild idx16/dst16 accordingly, so one program serves all cores."""
    npair = plans[0].npair
    kmax = {}
    for pr in range(npair):
        for gi in range(2):
            nb = len(plans[0].calls[pr][gi][1])
            kmax[(pr, gi)] = [max(p.calls[pr][gi][1][i] for p in plans)
                              for i in range(nb)]
    for p in plans:
        idx_vals, dst_vals, calls = [], [], []
        off = 0
        orig_iv = _unwrap_idx(p.idx16, p.totch)
        for pr in range(npair):
            groups = []
            for gi in range(2):
                base_id, ks = p.calls[pr][gi]
                kms = kmax[(pr, gi)]
                for i, (k, km) in enumerate(zip(ks, kms)):
                    iv = np.zeros(km * P, np.int16)
                    dv = np.full((P, km), -1, np.int16)
                    if k:
                        iv[:k * P] = orig_iv[off * P:(off + k) * P]
                        dv[:, :k] = p.dst16[:, off:off + k]
                    off += k
                    idx_vals.append(iv)
                    dst_vals.append(dv)
                groups.append((base_id, list(kms)))
            calls.append(groups)
        p.calls = calls
        p.totch = sum(sum(kmax[(pr, gi)]) for pr in range(npair) for gi in range(2))
        iv = np.concatenate(idx_vals) if idx_vals else np.zeros(0, np.int16)
        w = iv.reshape(-1, 16).T if iv.size else np.zeros((16, 0), np.int16)
        p.idx16 = np.tile(w, (8, 1)).copy()
        p.dst16 = np.concatenate(dst_vals, axis=1).copy() if dst_vals else \
            np.zeros((P, 1), np.int16)
    return plans


def _unwrap_idx(idx16, totch):
    """Inverse of the 16-partition wrap: [128, totch*8] -> flat [totch*128]."""
    if idx16.size == 0:
        return np.zeros(0, np.int16)
    return idx16[:16, :].T.reshape(-1)
